# revision 1
# baseline (speedup 1.0000x reference)
"""Distributed Trainium2 kernel for the gated-adapter attention module.

Head-parallel tensor parallelism over 8 NeuronCores (4 heads each):
wq/wk/wv sharded by output channels; attention computed per head in
S^T orientation (keys on partitions) so every matmul streams 512-wide;
an AllToAll converts the attention output from head-sharded to
token-sharded so each core applies the full wo projection to its
512-token chunk. All operand transposes go through the DMA xbar
(bf16) instead of the PE array. Compute bf16, f32 PSUM accumulation.
DRAM intermediates are split per-panel so the Tile scheduler can
overlap staging, compute, and spills across phases.
"""

import sys

sys.path.insert(0, "/opt/trn_rl_repo")

import numpy as np

import concourse.bass as bass
import concourse.mybir as mybir
import concourse.tile as tile
from concourse import bacc, bass_utils
from concourse.bass import ds, ts
from concourse.masks import make_identity

N_CORES = 8
B, S, D = 2, 2048, 4096
H = 32
HD = 128                      # head dim
H_LOC = H // N_CORES          # 4 heads per core
CH = H_LOC * HD               # 512 local channels
TOK = B * S                   # 4096 tokens
NK = D // 128                 # 32 contraction tiles
AL = 10                       # adapter length
TPC = TOK // N_CORES          # 512 tokens per core after AllToAll
NQC = S // 512                # 4 query chunks per sequence
NPAN = TOK // 512             # 8 token panels
SCALE = 1.0 / float(np.sqrt(HD))
BF = mybir.dt.bfloat16
F32 = mybir.dt.float32
EXP = mybir.ActivationFunctionType.Exp
TANH = mybir.ActivationFunctionType.Tanh
MULT = mybir.AluOpType.mult
ADD = mybir.AluOpType.add


def build():
    nc = bacc.Bacc("TRN2", target_bir_lowering=False, debug=False,
                   num_devices=N_CORES)
    x = nc.dram_tensor("x", [TOK, D], F32, kind="ExternalInput")
    wq = nc.dram_tensor("wq", [CH, D], F32, kind="ExternalInput")
    wk = nc.dram_tensor("wk", [CH, D], F32, kind="ExternalInput")
    wv = nc.dram_tensor("wv", [CH, D], F32, kind="ExternalInput")
    wo = nc.dram_tensor("wo", [D, D], F32, kind="ExternalInput")
    gate = nc.dram_tensor("gate", [1, H_LOC], F32, kind="ExternalInput")
    adapter = nc.dram_tensor("adapter", [AL, D], F32, kind="ExternalInput")
    fcos = nc.dram_tensor("fcos", [S, HD // 2], F32, kind="ExternalInput")
    fsin = nc.dram_tensor("fsin", [S, HD // 2], F32, kind="ExternalInput")
    mask = nc.dram_tensor("mask", [S, S], F32, kind="ExternalInput")
    out = nc.dram_tensor("out", [TPC, D], F32, kind="ExternalOutput")

    with tile.TileContext(nc) as tc:
        with tc.tile_pool(name="dram", bufs=1, space="DRAM") as dram, \
             tc.tile_pool(name="persist", bufs=1) as persist:
            wb_ds = [dram.tile([CH, D], BF, tag=f"wb{i}", name=f"wb{i}")
                     for i in range(3)]
            wob_d = dram.tile([D, D], BF, tag="wob_d")
            woT_d = dram.tile([D, D], BF, tag="woT_d")
            qn_ds = [dram.tile([S, CH], BF, tag=f"qn{b}", name=f"qn{b}")
                     for b in range(B)]
            kn_ds = [dram.tile([S, CH], BF, tag=f"kn{b}", name=f"kn{b}")
                     for b in range(B)]
            v_ds = [dram.tile([S, CH], BF, tag=f"vn{b}", name=f"vn{b}")
                    for b in range(B)]
            oT_d = dram.tile([CH, TOK], BF, tag="oT_d")
            a2a_in = dram.tile([N_CORES, CH, TPC], BF, tag="a2a_in")
            a2a_out = dram.tile([N_CORES, CH, TPC], BF, tag="a2a_out")

            ident = persist.tile([128, 128], BF, tag="ident")
            make_identity(nc, ident[:])
            ones = persist.tile([128, 128], BF, tag="ones")
            nc.vector.memset(ones[:], 1.0)
            g_sb = persist.tile([128, H_LOC], F32, tag="g_sb")
            g_in = persist.tile([128, H_LOC], F32, tag="g_in")
            nc.scalar.dma_start(g_in[:], gate.ap().partition_broadcast(128))
            nc.scalar.activation(g_sb[:], g_in[:], TANH)
            a_kT = persist.tile([128, H_LOC, AL], BF, tag="a_kT")
            a_v = persist.tile([AL, H_LOC, HD], BF, tag="a_v")
            maskT = persist.tile([128, S // 128, 128], BF, tag="maskT")
            cs_all = persist.tile([128, S // 128, HD // 2], F32, tag="cs_all")
            nc.scalar.dma_start(
                cs_all[:], fcos.ap().rearrange("(pb p) f -> p pb f", p=128))
            sn_all = persist.tile([128, S // 128, HD // 2], F32, tag="sn_all")
            nc.scalar.dma_start(
                sn_all[:], fsin.ap().rearrange("(pb p) f -> p pb f", p=128))

            # ================= phase 1: QKV (single pass) =================
            with tc.tile_pool(name="wph", bufs=1) as wph, \
                 tc.tile_pool(name="pst", bufs=2, space="PSUM") as pst, \
                 tc.tile_pool(name="psb", bufs=2, space="PSUM") as psb:
                aT = persist.tile([128, NK, AL], BF, tag="aT")
                with tc.tile_pool(name="stg", bufs=2) as stg:
                    # mask^T diagonal blocks (PE transpose, bf16)
                    for dblk in range(S // 128):
                        mdf = stg.tile([128, 128], F32, tag="mdf")
                        nc.scalar.dma_start(
                            mdf[:], mask.ap()[ts(dblk, 128), ts(dblk, 128)])
                        mdb = stg.tile([128, 128], BF, tag="mdb")
                        nc.vector.tensor_copy(mdb[:], mdf[:])
                        mps = pst.tile([128, 128], BF, tag="mps")
                        nc.tensor.transpose(mps[:], mdb[:], ident[:])
                        nc.vector.tensor_copy(maskT[:, dblk, :], mps[:])
                    # adapter^T [128 dim, AL] tiles (PE transpose, bf16)
                    ab = stg.tile([AL, D], BF, tag="ab", bufs=1)
                    for hf in range(4):
                        af = stg.tile([AL, D // 4], F32, tag="af")
                        nc.scalar.dma_start(af[:],
                                            adapter.ap()[:, ts(hf, D // 4)])
                        nc.vector.tensor_copy(ab[:, ts(hf, D // 4)], af[:])
                    for dt in range(NK):
                        aps = pst.tile([128, 128], BF, tag="mps")
                        nc.tensor.transpose(aps[:, :AL], ab[:, ts(dt, 128)],
                                            ident[:AL, :AL])
                        nc.vector.tensor_copy(aT[:, dt, :], aps[:, :AL])
                    # stage bf16 copies of wq/wk/wv in DRAM
                    for p_i, wt in ((0, wq), (1, wk), (2, wv)):
                        for cs in range(CH // 128):
                            for hf in range(2):
                                wf = stg.tile([128, D // 2], F32, tag="wf",
                                              bufs=2)
                                wbt = stg.tile([128, D // 2], BF, tag="wbt",
                                               bufs=2)
                                nc.scalar.dma_start(
                                    wf[:], wt.ap()[ts(cs, 128), ts(hf, D // 2)])
                                nc.vector.tensor_copy(wbt[:], wf[:])
                                nc.sync.dma_start(
                                    wb_ds[p_i][ts(cs, 128), ts(hf, D // 2)],
                                    wbt[:])
                    # stage x as bf16, one DRAM tile per 512-token panel
                # load all three transposed weight sets [128, NK, CH]
                wTs = []
                for p_i in range(3):
                    wT = wph.tile([128, NK, CH], BF, tag=f"wT{p_i}",
                                  name=f"wT{p_i}")
                    wTs.append(wT)
                    for dt in range(NK):
                        nc.sync.dma_start_transpose(
                            wT[:, dt, :], wb_ds[p_i][:, ts(dt, 128)])
                # a_k^T [ch, AL] per head, a_v [AL, ch]
                for cs in range(H_LOC):
                    pk = psb.tile([128, CH], F32, tag="ppq")
                    for dt in range(NK):
                        nc.tensor.matmul(pk[:, :AL],
                                         lhsT=wTs[1][:, dt, ts(cs, 128)],
                                         rhs=aT[:, dt, :], start=(dt == 0),
                                         stop=(dt == NK - 1))
                    nc.vector.tensor_copy(a_kT[:, cs, :], pk[:, :AL])
                pv = psb.tile([128, CH], F32, tag="ppq")
                for dt in range(NK):
                    nc.tensor.matmul(pv[:AL, :], lhsT=aT[:, dt, :],
                                     rhs=wTs[2][:, dt, :], start=(dt == 0),
                                     stop=(dt == NK - 1))
                for cs in range(H_LOC):
                    nc.vector.tensor_copy(a_v[:, cs, :], pv[:AL, ts(cs, 128)])

                # main QKV: quarter-panels of 512 tokens
                with tc.tile_pool(name="run", bufs=2) as st:
                    for qp in range(NPAN):
                        b_i, prow = qp // NQC, (qp % NQC) * 512
                        # load + cast this panel, transpose on the PE
                        xT = st.tile([128, NK, 512], BF, tag="xT")
                        for sp_i in range(4):
                            tstr = qp * 4 + sp_i
                            for hf in range(4):
                                xf = st.tile([128, D // 4], F32, tag="xf",
                                             bufs=2)
                                xbt = st.tile([128, D // 4], BF, tag="xbt",
                                              bufs=2)
                                nc.scalar.dma_start(
                                    xf[:],
                                    x.ap()[ts(tstr, 128), ts(hf, D // 4)])
                                nc.vector.tensor_copy(xbt[:], xf[:])
                                for dtl in range(NK // 4):
                                    dt = hf * (NK // 4) + dtl
                                    xtp = pst.tile([128, 128], BF, tag="mps")
                                    nc.tensor.transpose(
                                        xtp[:], xbt[:, ts(dtl, 128)], ident[:])
                                    nc.vector.tensor_copy(
                                        xT[:, dt, ts(sp_i, 128)], xtp[:])
                        # wo staging rides along, one eighth per panel
                        for dstr in range(qp * 4, qp * 4 + 4):
                            for hf in range(4):
                                wof = st.tile([128, D // 4], F32, tag="wof",
                                              bufs=2)
                                wob = st.tile([128, D // 4], BF, tag="wob",
                                              bufs=2)
                                nc.scalar.dma_start(
                                    wof[:],
                                    wo.ap()[ts(dstr, 128), ts(hf, D // 4)])
                                nc.vector.tensor_copy(wob[:], wof[:])
                                nc.sync.dma_start(
                                    wob_d[ts(dstr, 128), ts(hf, D // 4)],
                                    wob[:])
                        for sp_i in range(4):
                            srow = prow + sp_i * 128
                            pps = [psb.tile([128, CH], F32, tag=f"pp{pn}",
                                            name=f"pp{pn}") for pn in "qkv"]
                            for dt in range(NK):
                                for p_i in range(3):
                                    nc.tensor.matmul(
                                        pps[p_i][:],
                                        lhsT=xT[:, dt, ts(sp_i, 128)],
                                        rhs=wTs[p_i][:, dt, :],
                                        start=(dt == 0), stop=(dt == NK - 1))
                            # v: plain cast+store
                            vb = st.tile([128, CH], BF, tag="vb")
                            nc.vector.tensor_copy(vb[:], pps[2][:])
                            nc.scalar.dma_start(v_ds[b_i][ds(srow, 128), :],
                                                vb[:])
                            # q, k: RoPE then store natural
                            csb = cs_all[:, (srow // 128) % (S // 128), :]
                            ssb = sn_all[:, (srow // 128) % (S // 128), :]
                            for p_i, dstl in ((0, qn_ds), (1, kn_ds)):
                                rp = st.tile([128, CH], BF, tag=f"rp{p_i}",
                                             name=f"rp{p_i}")
                                for h in range(H_LOC):
                                    pv2 = pps[p_i][:, ts(h, HD)].rearrange(
                                        "p (i two) -> p two i", two=2)
                                    rv = rp[:, ts(h, HD)].rearrange(
                                        "p (i two) -> p two i", two=2)
                                    a0, b0 = pv2[:, 0, :], pv2[:, 1, :]
                                    t1 = st.tile([128, HD // 2], F32, tag="t1")
                                    t2 = st.tile([128, HD // 2], F32, tag="t2")
                                    nc.vector.tensor_mul(t1[:], a0, csb)
                                    nc.vector.tensor_mul(t2[:], b0, ssb)
                                    nc.vector.tensor_sub(rv[:, 0, :],
                                                         t1[:], t2[:])
                                    nc.vector.tensor_mul(t1[:], a0, ssb)
                                    nc.vector.tensor_mul(t2[:], b0, csb)
                                    nc.vector.tensor_add(rv[:, 1, :],
                                                         t1[:], t2[:])
                                nc.scalar.dma_start(
                                    dstl[b_i][ds(srow, 128), :], rp[:])

            # ========== phase 2: attention (+ wo transpose in gaps) ==========
            with tc.tile_pool(name="at", bufs=2) as at, \
                 tc.tile_pool(name="att", bufs=3) as att, \
                 tc.tile_pool(name="ps_st", bufs=3, space="PSUM") as ps_st, \
                 tc.tile_pool(name="ps_ac", bufs=1, space="PSUM") as ps_ac:
                def _bh_loads(b_i, h):
                    qTb = at.tile([128, S], BF, tag="qTb", name="qTb")
                    nc.sync.dma_start_transpose(
                        qTb[:], qn_ds[b_i][:, ts(h, HD)])
                    kTb = at.tile([128, S], BF, tag="kTb", name="kTb")
                    nc.sync.dma_start_transpose(
                        kTb[:], kn_ds[b_i][:, ts(h, HD)])
                    vb2 = at.tile([128, S // 128, HD], BF, tag="vb2",
                                  name="vb2")
                    nc.scalar.dma_start(
                        vb2[:],
                        v_ds[b_i][:, ts(h, HD)].rearrange(
                            "(kt p) d -> p kt d", p=128))
                    return qTb, kTb, vb2

                cur = _bh_loads(0, 0)
                for bh in range(B * H_LOC):
                    b_i, h = divmod(bh, H_LOC)
                    if True:
                        nxt = (_bh_loads(*divmod(bh + 1, H_LOC))
                               if bh + 1 < B * H_LOC else None)
                        qTb, kTb, vb2 = cur
                        for qc in range(NQC):
                            nkt = (qc + 1) * 4
                            stb = att.tile([128, S // 128, 512], BF, tag="stb",
                                           bufs=2)
                            for kt in range(nkt):
                                sps = ps_st.tile([128, 512], F32, tag="sps")
                                nc.tensor.matmul(sps[:],
                                                 lhsT=kTb[:, ts(kt, 128)],
                                                 rhs=qTb[:, ts(qc, 512)],
                                                 start=True, stop=True)
                                if kt // 4 == qc:
                                    off = (kt % 4) * 128
                                    if off > 0:
                                        nc.vector.memset(
                                            stb[:, kt, ds(0, off)], 0.0)
                                    sd = att.tile([128, 128], F32, tag="sd")
                                    nc.vector.scalar_tensor_tensor(
                                        sd[:], sps[:, ds(off, 128)], SCALE,
                                        maskT[:, kt, :], op0=MULT, op1=ADD)
                                    nc.scalar.activation(
                                        stb[:, kt, ds(off, 128)], sd[:], EXP)
                                    if off + 128 < 512:
                                        nc.scalar.activation(
                                            stb[:, kt,
                                                ds(off + 128, 384 - off)],
                                            sps[:, ds(off + 128, 384 - off)],
                                            EXP, scale=SCALE)
                                else:
                                    nc.scalar.activation(stb[:, kt, :], sps[:],
                                                         EXP, scale=SCALE)
                            # adapter scores [AL, 512]
                            spa = ps_st.tile([128, 512], F32, tag="sps")
                            nc.tensor.matmul(spa[:AL, :], lhsT=a_kT[:, h, :],
                                             rhs=qTb[:, ts(qc, 512)],
                                             start=True, stop=True)
                            pab = att.tile([AL, 512], BF, tag="pab")
                            nc.scalar.activation(pab[:], spa[:AL, :], EXP,
                                                 scale=SCALE)
                            # column sums via ones-matmul
                            s_ps = ps_ac.tile([1, 512], F32, tag="s_ps")
                            sa_ps = ps_ac.tile([1, 512], F32, tag="sa_ps")
                            for kt in range(nkt):
                                nc.tensor.matmul(s_ps[:], lhsT=ones[:, 0:1],
                                                 rhs=stb[:, kt, :],
                                                 start=(kt == 0),
                                                 stop=(kt == nkt - 1))
                            nc.tensor.matmul(sa_ps[:], lhsT=ones[:AL, 0:1],
                                             rhs=pab[:], start=True, stop=True)
                            # PV accumulation: oT [128 d, 512 q]
                            o_ps = ps_ac.tile([128, 512], F32, tag="o_ps", bufs=2)
                            for kt in range(nkt):
                                nc.tensor.matmul(o_ps[:], lhsT=vb2[:, kt, :],
                                                 rhs=stb[:, kt, :],
                                                 start=(kt == 0),
                                                 stop=(kt == nkt - 1))
                            oa_ps = ps_ac.tile([128, 512], F32, tag="oa_ps")
                            nc.tensor.matmul(oa_ps[:], lhsT=a_v[:, h, :],
                                             rhs=pab[:], start=True, stop=True)
                            # combine: o = o_main/s_main + tanh(g)*oa/s_adapt
                            sb2 = att.tile([1, 512], BF, tag="sb2")
                            nc.vector.tensor_copy(sb2[:], s_ps[:])
                            sb2a = att.tile([1, 512], BF, tag="sb2a")
                            nc.vector.tensor_copy(sb2a[:], sa_ps[:])
                            bc_ps = ps_st.tile([128, 512], F32, tag="sps")
                            nc.tensor.matmul(bc_ps[:], lhsT=ones[0:1, :],
                                             rhs=sb2[:], start=True, stop=True)
                            bca_ps = ps_st.tile([128, 512], F32, tag="sps")
                            nc.tensor.matmul(bca_ps[:], lhsT=ones[0:1, :],
                                             rhs=sb2a[:], start=True, stop=True)
                            rb = att.tile([128, 512], F32, tag="rb")
                            nc.vector.reciprocal_approx_fast(rb[:], bc_ps[:])
                            rba = att.tile([128, 512], F32, tag="rba")
                            nc.vector.reciprocal_approx_fast(rba[:], bca_ps[:])
                            t3 = att.tile([128, 512], F32, tag="t3")
                            nc.vector.tensor_mul(t3[:], o_ps[:], rb[:])
                            t4 = att.tile([128, 512], F32, tag="t4")
                            nc.vector.scalar_tensor_tensor(
                                t4[:], rba[:], g_sb[:, ds(h, 1)], oa_ps[:],
                                op0=MULT, op1=MULT)
                            ob = att.tile([128, 512], BF, tag="ob")
                            nc.vector.tensor_add(ob[:], t3[:], t4[:])
                            nc.scalar.dma_start(
                                oT_d[ts(h, HD),
                                     ds(b_i * S + qc * 512, 512)], ob[:])
                        # wo transpose chunks slotted into attention downtime
                        if bh >= 3:
                            for et in range((bh - 3) * 7,
                                            min(32, (bh - 3) * 7 + 7)):
                                wot_b = at.tile([128, D], BF, tag="wot_b",
                                                name="wot_b")
                                nc.sync.dma_start_transpose(
                                    wot_b[:], wob_d[:, ts(et, 128)])
                                nc.sync.dma_start(woT_d[ts(et, 128), :],
                                                    wot_b[:])
                        cur = nxt

            # ================= phase 3: AllToAll + wo =================
            for j in range(N_CORES):
                nc.scalar.dma_start(a2a_in[j], oT_d[:, ds(j * TPC, TPC)])
            nc.gpsimd.collective_compute(
                "AllToAll", mybir.AluOpType.bypass,
                replica_groups=[list(range(N_CORES))],
                ins=[a2a_in.opt()], outs=[a2a_out.opt()])
            with tc.tile_pool(name="wo_sb", bufs=3) as wsb, \
                 tc.tile_pool(name="wo_ps", bufs=1, space="PSUM") as wps, \
                 tc.tile_pool(name="of", bufs=1) as ofp:
                oTf = ofp.tile([128, NK, TPC], BF, tag="oTf")
                for sc in range(N_CORES):
                    nc.scalar.dma_start(
                        oTf[:, ds(sc * H_LOC, H_LOC), :],
                        a2a_out[sc].rearrange("(g p) t -> p g t", p=128))
                # 4 passes over d (1024 cols each); 8 psum banks = 4 tt x 2 d2
                for dp in range(4):
                    yps = [wps.tile([128, 512], F32, tag=f"yp{i}",
                                    name=f"yp{i}") for i in range(8)]
                    for et in range(NK):
                        wot = wsb.tile([128, 1024], BF, tag="wot")
                        nc.scalar.dma_start(
                            wot[:], woT_d[ts(et, 128), ts(dp, 1024)])
                        for tt in range(TPC // 128):
                            for d2 in range(2):
                                nc.tensor.matmul(
                                    yps[tt * 2 + d2][:],
                                    lhsT=oTf[:, et, ts(tt, 128)],
                                    rhs=wot[:, ts(d2, 512)],
                                    start=(et == 0), stop=(et == NK - 1))
                    for tt in range(TPC // 128):
                        for d2 in range(2):
                            yb = wsb.tile([128, 512], F32, tag="yb")
                            nc.vector.tensor_copy(yb[:], yps[tt * 2 + d2][:])
                            nc.scalar.dma_start(
                                out.ap()[ts(tt, 128),
                                         ds(dp * 1024 + d2 * 512, 512)],
                                yb[:])
    nc.compile()
    return nc


_NC_CACHE = None


def kernel(x, wq, wk, wv, wo, gate, adapter, freqs_cos, freqs_sin, mask,
           start_pos=0, **_unused):
    global _NC_CACHE
    if _NC_CACHE is None:
        _NC_CACHE = build()
    nc = _NC_CACHE
    xf = np.ascontiguousarray(np.asarray(x, np.float32).reshape(TOK, D))
    g = np.asarray(gate, np.float32).reshape(H)
    in_maps = []
    for r in range(N_CORES):
        sl = slice(r * CH, (r + 1) * CH)
        in_maps.append({
            "x": xf,
            "wq": np.ascontiguousarray(np.asarray(wq, np.float32)[sl]),
            "wk": np.ascontiguousarray(np.asarray(wk, np.float32)[sl]),
            "wv": np.ascontiguousarray(np.asarray(wv, np.float32)[sl]),
            "wo": np.ascontiguousarray(np.asarray(wo, np.float32)),
            "gate": np.ascontiguousarray(
                g[r * H_LOC:(r + 1) * H_LOC].reshape(1, H_LOC)),
            "adapter": np.ascontiguousarray(
                np.asarray(adapter, np.float32).reshape(AL, D)),
            "fcos": np.ascontiguousarray(np.asarray(freqs_cos, np.float32)),
            "fsin": np.ascontiguousarray(np.asarray(freqs_sin, np.float32)),
            "mask": np.ascontiguousarray(
                np.asarray(mask, np.float32).reshape(S, S)),
        })
    res = bass_utils.run_bass_kernel_spmd(nc, in_maps,
                                          core_ids=list(range(N_CORES)))
    y = np.concatenate([res.results[r]["out"] for r in range(N_CORES)], axis=0)
    return y.reshape(B, S, D)


if __name__ == "__main__":
    nc = build()
    print("compiled ok, instrs:",
          sum(len(bb.instructions) for f in nc.m.functions for bb in f.blocks))



# revision 8
# speedup vs baseline: 1.1862x; 1.1862x over previous
"""Distributed Trainium2 kernel for the gated-adapter attention module.

Head-parallel tensor parallelism over 8 NeuronCores (4 heads each).
Host-side prep (inside kernel()): inputs are pre-transposed and
pre-cast to bf16 so the device never transposes weights or x —
xT [D, TOK], wqT/wkT/wvT [D, CH], and the core's wo column-slice
woT [CH, D] arrive matmul-ready.  The adapter K/V projections
(10x4096 @ 4096x512) and tanh(gate) are precomputed on host.

Device pipeline per core:
  A) QKV: x-stationary matmuls emit q/k/v natural [tok, ch]; RoPE on
     DVE straight out of PSUM; q/k PE-transposed to [ch, tok] and
     spilled to DRAM (contiguous loads later), v spilled natural.
  B) Attention per (batch, head) in S^T orientation: scores [k, q],
     uniform exp on ACT, multiplicative 0/1 diagonal masks on DVE,
     softmax sums via ones-matmul, PV accumulation, gated adapter
     branch; output o^T accumulates in SBUF (no spill).
  C) RowParallel wo: y_partial[tok, dout] = o^T.T @ woT with woT
     SBUF-resident, chunked over 4 dout-quarters; each quarter's bf16
     partial goes through an 8-core ReduceScatter (pipelined against
     the next quarter's matmuls); final f32 cast + store.
"""

import sys

sys.path.insert(0, "/opt/trn_rl_repo")

import numpy as np
import ml_dtypes

import concourse.bass as bass
import concourse.mybir as mybir
import concourse.tile as tile
from concourse import bacc, bass_utils
from concourse.bass import ds, ts
from concourse.masks import make_identity

N_CORES = 8
B, S, D = 2, 2048, 4096
H = 32
HD = 128                      # head dim
H_LOC = H // N_CORES          # 4 heads per core
CH = H_LOC * HD               # 512 local channels
TOK = B * S                   # 4096 tokens
NK = D // 128                 # 32 contraction tiles
AL = 10                       # adapter length
TPC = TOK // N_CORES          # 512 tokens per core after ReduceScatter
NQC = S // 512                # 4 query chunks per sequence
NPAN = TOK // 512             # 8 token panels
NDQ = 4                       # dout quarters for the wo/RS pipeline
DQW = D // NDQ                # 1024 cols per quarter
SCALE = 1.0 / float(np.sqrt(HD))
BF = mybir.dt.bfloat16
F32 = mybir.dt.float32
EXP = mybir.ActivationFunctionType.Exp
COPY = mybir.ActivationFunctionType.Copy
MULT = mybir.AluOpType.mult
BF_NP = ml_dtypes.bfloat16


def build():
    nc = bacc.Bacc("TRN2", target_bir_lowering=False, debug=False,
                   num_devices=N_CORES)
    xT = nc.dram_tensor("xT", [D, TOK], BF, kind="ExternalInput")
    wqT = nc.dram_tensor("wqT", [D, CH], BF, kind="ExternalInput")
    wkT = nc.dram_tensor("wkT", [D, CH], BF, kind="ExternalInput")
    wvT = nc.dram_tensor("wvT", [D, CH], BF, kind="ExternalInput")
    woT = nc.dram_tensor("woT", [CH, D], BF, kind="ExternalInput")
    fcos = nc.dram_tensor("fcos", [S, HD // 2], F32, kind="ExternalInput")
    fsin = nc.dram_tensor("fsin", [S, HD // 2], F32, kind="ExternalInput")
    akT = nc.dram_tensor("akT", [HD, H_LOC * AL], BF, kind="ExternalInput")
    av = nc.dram_tensor("av", [AL, CH], BF, kind="ExternalInput")
    tg = nc.dram_tensor("tg", [1, H_LOC], F32, kind="ExternalInput")
    m01 = nc.dram_tensor("m01", [128, 4 * 512], BF, kind="ExternalInput")
    out = nc.dram_tensor("out", [TPC, D], F32, kind="ExternalOutput")

    with tile.TileContext(nc) as tc:
        with tc.tile_pool(name="dram", bufs=1, space="DRAM") as dram, \
             tc.tile_pool(name="persist", bufs=1) as persist:
            qT_d = dram.tile([CH, TOK], BF, tag="qT_d")
            kT_d = dram.tile([CH, TOK], BF, tag="kT_d")
            v_d = dram.tile([TOK, CH], BF, tag="v_d")
            y_ds = [dram.tile([TOK, DQW], BF, tag=f"y{i}", name=f"y{i}")
                    for i in range(NDQ)]
            yr_ds = [dram.tile([TPC, DQW], BF, tag=f"yr{i}", name=f"yr{i}")
                     for i in range(NDQ)]

            ident = persist.tile([128, 128], BF, tag="ident")
            make_identity(nc, ident[:])
            ones = persist.tile([128, 128], BF, tag="ones")
            nc.vector.memset(ones[:], 1.0)
            tg_sb = persist.tile([128, H_LOC], F32, tag="tg_sb")
            nc.scalar.dma_start(tg_sb[:], tg.ap().partition_broadcast(128))
            akT_sb = persist.tile([128, H_LOC, AL], BF, tag="akT_sb")
            nc.scalar.dma_start(
                akT_sb[:], akT.ap().rearrange("p (h a) -> p h a", h=H_LOC))
            av_sb = persist.tile([AL, CH], BF, tag="av_sb")
            nc.scalar.dma_start(av_sb[:], av.ap())
            m01_sb = persist.tile([128, 4, 512], BF, tag="m01_sb")
            nc.scalar.dma_start(
                m01_sb[:], m01.ap().rearrange("p (j q) -> p j q", j=4))
            # RoPE tables in the baseline layout: [128 part, S//128, 64]
            cs_all = persist.tile([128, S // 128, HD // 2], F32, tag="cs_all")
            nc.scalar.dma_start(
                cs_all[:], fcos.ap().rearrange("(pb p) f -> p pb f", p=128))
            sn_all = persist.tile([128, S // 128, HD // 2], F32, tag="sn_all")
            nc.scalar.dma_start(
                sn_all[:], fsin.ap().rearrange("(pb p) f -> p pb f", p=128))

            # ================= phase A: QKV =================
            NPA = 16                  # 256-token panels
            with tc.tile_pool(name="wres", bufs=1) as wres, \
                 tc.tile_pool(name="xa", bufs=6) as xa, \
                 tc.tile_pool(name="ar", bufs=3) as ar, \
                 tc.tile_pool(name="aspill", bufs=2) as aspill, \
                 tc.tile_pool(name="ps_a", bufs=2, space="PSUM") as ps_a, \
                 tc.tile_pool(name="ps_t", bufs=2, space="PSUM") as ps_t:
                wTs = []
                for nm, wt in (("q", wqT), ("k", wkT), ("v", wvT)):
                    wT = wres.tile([128, NK, CH], BF, tag=f"wT{nm}",
                                   name=f"wT{nm}")
                    nc.sync.dma_start(
                        wT[:], wt.ap().rearrange("(nk p) c -> p nk c", p=128))
                    wTs.append(wT)

                for pan in range(NPA):
                    # xT quarter-tiles for this panel: [128, 8, 256] each
                    xh = []
                    for qd in range(4):
                        xt = xa.tile([128, NK // 4, 256], BF, tag="xt")
                        nc.sync.dma_start(
                            xt[:],
                            xT.ap()[ds(qd * (D // 4), D // 4),
                                    ds(pan * 256, 256)].rearrange(
                                "(k p) t -> p k t", p=128))
                        xh.append(xt)
                    pps = [[ps_a.tile([128, CH], F32, tag=f"pp{nm}",
                                      name=f"pp{nm}") for nm in "qkv"]
                           for _ in range(2)]
                    for dt in range(NK):
                        for ck in range(2):
                            lx = xh[dt // (NK // 4)][:, dt % (NK // 4),
                                                     ts(ck, 128)]
                            for p_i in range(3):
                                nc.tensor.matmul(
                                    pps[ck][p_i][:], lhsT=lx,
                                    rhs=wTs[p_i][:, dt, :],
                                    start=(dt == 0), stop=(dt == NK - 1))
                    for ck in range(2):          # 128-token chunks in panel
                        tglob = pan * 2 + ck     # global token tile
                        srow = (tglob % (S // 128)) * 128  # within batch
                        # v: cast + spill natural
                        vb = ar.tile([128, CH], BF, tag="vb")
                        nc.scalar.activation(vb[:], pps[ck][2][:], COPY)
                        nc.sync.dma_start(v_d[ds(tglob * 128, 128), :], vb[:])
                        # q, k: RoPE from PSUM -> natural bf16
                        csb = cs_all[:, srow // 128, :]
                        ssb = sn_all[:, srow // 128, :]
                        for p_i, dst in ((0, qT_d), (1, kT_d)):
                            rp = ar.tile([128, CH], BF, tag=f"rp{p_i}",
                                         name=f"rp{p_i}")
                            for h in range(H_LOC):
                                pv2 = pps[ck][p_i][:, ts(h, HD)].rearrange(
                                    "p (i two) -> p two i", two=2)
                                rv = rp[:, ts(h, HD)].rearrange(
                                    "p (i two) -> p two i", two=2)
                                a0, b0 = pv2[:, 0, :], pv2[:, 1, :]
                                t1 = ar.tile([128, HD // 2], F32, tag="t1")
                                t2 = ar.tile([128, HD // 2], F32, tag="t2")
                                nc.vector.tensor_mul(t1[:], a0, csb)
                                nc.vector.tensor_mul(t2[:], b0, ssb)
                                nc.vector.tensor_sub(rv[:, 0, :], t1[:], t2[:])
                                nc.vector.tensor_mul(t1[:], a0, ssb)
                                nc.vector.tensor_mul(t2[:], b0, csb)
                                nc.vector.tensor_add(rv[:, 1, :], t1[:], t2[:])
                            # transpose each 128-ch block to [ch, tok]
                            for ct in range(4):
                                tp = ps_t.tile([128, 128], BF, tag="tp")
                                nc.tensor.transpose(tp[:], rp[:, ts(ct, 128)],
                                                    ident[:])
                                st = aspill.tile([128, 128], BF, tag="st",
                                                 bufs=4)
                                nc.scalar.activation(st[:], tp[:], COPY)
                                nc.sync.dma_start(
                                    dst[ds(ct * 128, 128),
                                        ds(tglob * 128, 128)], st[:])

            # ================= phase B: attention =================
            obc_cm = tc.tile_pool(name="obc", bufs=1)
            obc = obc_cm.__enter__()
            woT_sb = obc.tile([128, H_LOC, D], BF, tag="woT_sb")
            nc.sync.dma_start(
                woT_sb[:], woT.ap().rearrange("(c p) d -> p c d", p=128))
            oT_sb = obc.tile([128, H_LOC, TOK], BF, tag="oT_sb")
            with tc.tile_pool(name="at", bufs=3) as at, \
                 tc.tile_pool(name="att", bufs=3) as att, \
                 tc.tile_pool(name="ps_st", bufs=3, space="PSUM") as ps_st, \
                 tc.tile_pool(name="ps_ac", bufs=1, space="PSUM") as ps_ac:
                def _bh_loads(b_i, h):
                    qTb = at.tile([128, S], BF, tag="qTb", name="qTb")
                    nc.sync.dma_start(
                        qTb[:], qT_d[ds(h * HD, HD), ds(b_i * S, S)])
                    kTb = at.tile([128, S], BF, tag="kTb", name="kTb")
                    nc.sync.dma_start(
                        kTb[:], kT_d[ds(h * HD, HD), ds(b_i * S, S)])
                    vb2 = at.tile([128, S // 128, HD], BF, tag="vb2",
                                  name="vb2")
                    nc.scalar.dma_start(
                        vb2[:],
                        v_d[ds(b_i * S, S), ts(h, HD)].rearrange(
                            "(kt p) d -> p kt d", p=128))
                    return qTb, kTb, vb2

                cur = _bh_loads(0, 0)
                for bh in range(B * H_LOC):
                    b_i, h = divmod(bh, H_LOC)
                    nxt = (_bh_loads(*divmod(bh + 1, H_LOC))
                           if bh + 1 < B * H_LOC else None)
                    qTb, kTb, vb2 = cur
                    for qc in range(NQC):
                        nkt = (qc + 1) * 4
                        stb = att.tile([128, S // 128, 512], BF, tag="stb",
                                       bufs=2)
                        for kt in range(nkt):
                            sps = ps_st.tile([128, 512], F32, tag="sps")
                            nc.tensor.matmul(sps[:],
                                             lhsT=kTb[:, ts(kt, 128)],
                                             rhs=qTb[:, ts(qc, 512)],
                                             start=True, stop=True)
                            nc.scalar.activation(stb[:, kt, :], sps[:],
                                                 EXP, scale=SCALE)
                            if kt // 4 == qc:
                                # diagonal block: multiplicative 0/1 mask
                                nc.vector.tensor_mul(
                                    stb[:, kt, :], stb[:, kt, :],
                                    m01_sb[:, kt % 4, :])
                        # adapter scores [AL, 512]
                        spa = ps_st.tile([128, 512], F32, tag="sps")
                        nc.tensor.matmul(spa[:AL, :], lhsT=akT_sb[:, h, :],
                                         rhs=qTb[:, ts(qc, 512)],
                                         start=True, stop=True)
                        pab = att.tile([AL, 512], BF, tag="pab")
                        nc.scalar.activation(pab[:], spa[:AL, :], EXP,
                                             scale=SCALE)
                        # column sums via ones-matmul
                        s_ps = ps_ac.tile([1, 512], F32, tag="s_ps")
                        sa_ps = ps_ac.tile([1, 512], F32, tag="sa_ps")
                        for kt in range(nkt):
                            nc.tensor.matmul(s_ps[:], lhsT=ones[:, 0:1],
                                             rhs=stb[:, kt, :],
                                             start=(kt == 0),
                                             stop=(kt == nkt - 1))
                        nc.tensor.matmul(sa_ps[:], lhsT=ones[:AL, 0:1],
                                         rhs=pab[:], start=True, stop=True)
                        # PV accumulation: oT [128 d, 512 q]
                        o_ps = ps_ac.tile([128, 512], F32, tag="o_ps", bufs=2)
                        for kt in range(nkt):
                            nc.tensor.matmul(o_ps[:], lhsT=vb2[:, kt, :],
                                             rhs=stb[:, kt, :],
                                             start=(kt == 0),
                                             stop=(kt == nkt - 1))
                        oa_ps = ps_ac.tile([128, 512], F32, tag="oa_ps")
                        nc.tensor.matmul(oa_ps[:], lhsT=av_sb[:, ts(h, HD)],
                                         rhs=pab[:], start=True, stop=True)
                        # combine: o = o_main/s_main + tanh(g)*oa/s_adapt
                        sb2 = att.tile([1, 512], BF, tag="sb2")
                        nc.vector.tensor_copy(sb2[:], s_ps[:])
                        sb2a = att.tile([1, 512], BF, tag="sb2a")
                        nc.vector.tensor_copy(sb2a[:], sa_ps[:])
                        bc_ps = ps_st.tile([128, 512], F32, tag="sps")
                        nc.tensor.matmul(bc_ps[:], lhsT=ones[0:1, :],
                                         rhs=sb2[:], start=True, stop=True)
                        bca_ps = ps_st.tile([128, 512], F32, tag="sps")
                        nc.tensor.matmul(bca_ps[:], lhsT=ones[0:1, :],
                                         rhs=sb2a[:], start=True, stop=True)
                        rb = att.tile([128, 512], F32, tag="rb")
                        nc.vector.reciprocal_approx_fast(rb[:], bc_ps[:])
                        rba = att.tile([128, 512], F32, tag="rba")
                        nc.vector.reciprocal_approx_fast(rba[:], bca_ps[:])
                        t3 = att.tile([128, 512], F32, tag="t3")
                        nc.vector.tensor_mul(t3[:], o_ps[:], rb[:])
                        t4 = att.tile([128, 512], F32, tag="t4")
                        nc.vector.scalar_tensor_tensor(
                            t4[:], rba[:], tg_sb[:, ds(h, 1)], oa_ps[:],
                            op0=MULT, op1=MULT)
                        nc.vector.tensor_add(
                            oT_sb[:, h, ds(b_i * S + qc * 512, 512)],
                            t3[:], t4[:])
                    cur = nxt

            # ============ phase C: RowParallel wo + ReduceScatter ============
            with tc.tile_pool(name="wy", bufs=3) as wy, \
                 tc.tile_pool(name="wyo", bufs=2) as wyo, \
                 tc.tile_pool(name="ps_y", bufs=4, space="PSUM") as ps_y:
                for dq in range(NDQ):
                    for tt in range(TOK // 128):
                        yt = ps_y.tile([128, DQW], F32, tag="yt")
                        for ct in range(H_LOC):
                            for hw in range(DQW // 512):
                                nc.tensor.matmul(
                                    yt[:, ts(hw, 512)],
                                    lhsT=oT_sb[:, ct, ts(tt, 128)],
                                    rhs=woT_sb[:, ct,
                                               ds(dq * DQW + hw * 512, 512)],
                                    start=(ct == 0), stop=(ct == H_LOC - 1))
                        yb = wy.tile([128, DQW], BF, tag="yb")
                        nc.scalar.activation(yb[:], yt[:], COPY)
                        nc.sync.dma_start(y_ds[dq][ds(tt * 128, 128), :],
                                          yb[:])
                    nc.gpsimd.collective_compute(
                        "ReduceScatter", mybir.AluOpType.add,
                        replica_groups=[list(range(N_CORES))],
                        ins=[y_ds[dq].opt()], outs=[yr_ds[dq].opt()])
                    # load reduced shard, cast to f32, store to output
                    yrb = wyo.tile([128, TPC // 128, DQW], BF, tag="yrb")
                    nc.scalar.dma_start(
                        yrb[:], yr_ds[dq].rearrange("(n p) d -> p n d", p=128))
                    for n in range(TPC // 128):
                        yf = wyo.tile([128, DQW], F32, tag="yf")
                        nc.vector.tensor_copy(yf[:], yrb[:, n, :])
                        nc.sync.dma_start(
                            out.ap()[ds(n * 128, 128), ds(dq * DQW, DQW)],
                            yf[:])
            obc_cm.__exit__(None, None, None)
    nc.compile()
    return nc


_NC_CACHE = None


def _prep(x, wq, wk, wv, wo, gate, adapter, freqs_cos, freqs_sin, mask):
    """Host-side layout prep. Returns per-core input maps."""
    xf = np.asarray(x, np.float32).reshape(TOK, D)
    xT = np.ascontiguousarray(xf.T).astype(BF_NP)
    wq = np.asarray(wq, np.float32)
    wk = np.asarray(wk, np.float32)
    wv = np.asarray(wv, np.float32)
    wo = np.asarray(wo, np.float32)
    g = np.tanh(np.asarray(gate, np.float32).reshape(H))
    ad = np.asarray(adapter, np.float32).reshape(AL, D)
    a_k = ad @ wk.T          # [AL, H*HD]
    a_v = ad @ wv.T
    fc = np.ascontiguousarray(np.asarray(freqs_cos, np.float32))
    fs = np.ascontiguousarray(np.asarray(freqs_sin, np.float32))
    mk = np.asarray(mask, np.float32).reshape(S, S)
    # multiplicative 0/1 diagonal masks, S^T orientation: m01[j][k, q]
    m01 = np.empty((128, 4, 512), np.float32)
    for j in range(4):
        blk = mk[0:512, j * 128:(j + 1) * 128]    # [q, k] additive
        m01[:, j, :] = (blk == 0.0).T.astype(np.float32)
    m01 = np.ascontiguousarray(m01.reshape(128, 4 * 512)).astype(BF_NP)

    in_maps = []
    for r in range(N_CORES):
        sl = slice(r * CH, (r + 1) * CH)
        akr = a_k[:, sl]     # [AL, CH]
        akT = np.zeros((HD, H_LOC, AL), np.float32)
        for h in range(H_LOC):
            akT[:, h, :] = akr[:, h * HD:(h + 1) * HD].T
        in_maps.append({
            "xT": xT,
            "wqT": np.ascontiguousarray(wq[sl].T).astype(BF_NP),
            "wkT": np.ascontiguousarray(wk[sl].T).astype(BF_NP),
            "wvT": np.ascontiguousarray(wv[sl].T).astype(BF_NP),
            "woT": np.ascontiguousarray(wo[:, sl].T).astype(BF_NP),
            "fcos": fc,
            "fsin": fs,
            "akT": np.ascontiguousarray(
                akT.reshape(HD, H_LOC * AL)).astype(BF_NP),
            "av": np.ascontiguousarray(a_v[:, sl]).astype(BF_NP),
            "tg": np.ascontiguousarray(
                g[r * H_LOC:(r + 1) * H_LOC].reshape(1, H_LOC)),
            "m01": m01,
        })
    return in_maps


def kernel(x, wq, wk, wv, wo, gate, adapter, freqs_cos, freqs_sin, mask,
           start_pos=0, **_unused):
    global _NC_CACHE
    if _NC_CACHE is None:
        _NC_CACHE = build()
    nc = _NC_CACHE
    in_maps = _prep(x, wq, wk, wv, wo, gate, adapter,
                    freqs_cos, freqs_sin, mask)
    res = bass_utils.run_bass_kernel_spmd(nc, in_maps,
                                          core_ids=list(range(N_CORES)))
    y = np.concatenate([res.results[r]["out"] for r in range(N_CORES)], axis=0)
    return y.reshape(B, S, D)


if __name__ == "__main__":
    nc = build()
    print("compiled ok, instrs:",
          sum(len(bb.instructions) for f in nc.m.functions for bb in f.blocks))


# revision 11
# speedup vs baseline: 1.1990x; 1.0108x over previous
"""Distributed Trainium2 kernel for the gated-adapter attention module.

Head-parallel tensor parallelism over 8 NeuronCores (4 heads each).
Host-side prep (inside kernel()): inputs are pre-transposed and
pre-cast to bf16 so the device never transposes weights or x —
xT [D, TOK], wqT/wkT/wvT [D, CH], and the core's wo column-slice
woT [CH, D] arrive matmul-ready.  The adapter K/V projections
(10x4096 @ 4096x512) and tanh(gate) are precomputed on host.

Device pipeline per core:
  A) QKV: x-stationary matmuls emit q/k/v natural [tok, ch]; RoPE on
     DVE straight out of PSUM; q/k PE-transposed to [ch, tok] and
     spilled to DRAM (contiguous loads later), v spilled natural.
  B) Attention per (batch, head) in S^T orientation: scores [k, q],
     uniform exp on ACT, multiplicative 0/1 diagonal masks on DVE,
     4:1 DVE pre-reduction then softmax sums via ones-matmul, PV
     accumulation, gated adapter branch; o^T accumulates in SBUF.
  C) RowParallel wo in y^T orientation: yT[dout, tok] = woT.T @ o^T
     with woT stationary (8 PSUM banks per dout-tile), partials
     ReduceScattered over dout in 4 chunks (pipelined against the
     matmuls); final f32 via SWDGE cast-DMA; host re-transposes.

Engine map: sync = x prefetch only; scalar = weights + compute-paced
copies/spills/exp; gpsimd = phase-B loads + RS triggers + output
(fire on data semaphores, never queue behind compute).
"""

import sys

sys.path.insert(0, "/opt/trn_rl_repo")

import numpy as np
import ml_dtypes

import concourse.bass as bass
import concourse.mybir as mybir
import concourse.tile as tile
from concourse import bacc, bass_utils
from concourse.bass import ds, ts
from concourse.masks import make_identity

N_CORES = 8
B, S, D = 2, 2048, 4096
H = 32
HD = 128                      # head dim
H_LOC = H // N_CORES          # 4 heads per core
CH = H_LOC * HD               # 512 local channels
TOK = B * S                   # 4096 tokens
NK = D // 128                 # 32 contraction tiles
AL = 10                       # adapter length
NQC = S // 512                # 4 query chunks per sequence
NDQ = 4                       # dout chunks for the wo/RS pipeline
DQW = D // NDQ                # 1024 dout rows per chunk
SCALE = 1.0 / float(np.sqrt(HD))
BF = mybir.dt.bfloat16
F32 = mybir.dt.float32
EXP = mybir.ActivationFunctionType.Exp
COPY = mybir.ActivationFunctionType.Copy
MULT = mybir.AluOpType.mult
BF_NP = ml_dtypes.bfloat16


def build():
    nc = bacc.Bacc("TRN2", target_bir_lowering=False, debug=False,
                   num_devices=N_CORES)
    xT = nc.dram_tensor("xT", [D, TOK], BF, kind="ExternalInput")
    wqT = nc.dram_tensor("wqT", [D, CH], BF, kind="ExternalInput")
    wkT = nc.dram_tensor("wkT", [D, CH], BF, kind="ExternalInput")
    wvT = nc.dram_tensor("wvT", [D, CH], BF, kind="ExternalInput")
    woT = nc.dram_tensor("woT", [CH, D], BF, kind="ExternalInput")
    fcos = nc.dram_tensor("fcos", [S, HD // 2], F32, kind="ExternalInput")
    fsin = nc.dram_tensor("fsin", [S, HD // 2], F32, kind="ExternalInput")
    akT = nc.dram_tensor("akT", [HD, H_LOC * AL], BF, kind="ExternalInput")
    av = nc.dram_tensor("av", [AL, CH], BF, kind="ExternalInput")
    tg = nc.dram_tensor("tg", [1, H_LOC], F32, kind="ExternalInput")
    m01 = nc.dram_tensor("m01", [128, 4 * 512], BF, kind="ExternalInput")
    # out holds yT shards: rows (dq*128+p) = summed yT row dq*1024+r*128+p
    out = nc.dram_tensor("out", [TOK // N_CORES, TOK], F32,
                         kind="ExternalOutput")

    with tile.TileContext(nc) as tc:
        with tc.tile_pool(name="dram", bufs=1, space="DRAM") as dram, \
             tc.tile_pool(name="persist", bufs=1) as persist, \
             tc.tile_pool(name="at", bufs=3) as at:
            qT_d = dram.tile([CH, TOK], BF, tag="qT_d")
            kT_d = dram.tile([CH, TOK], BF, tag="kT_d")
            v_d = dram.tile([TOK, CH], BF, tag="v_d")
            yT_ds = [dram.tile([DQW, TOK], BF, tag=f"yT{i}", name=f"yT{i}")
                     for i in range(NDQ)]
            yr_ds = [dram.tile([DQW // N_CORES, TOK], BF, tag=f"yr{i}",
                               name=f"yr{i}") for i in range(NDQ)]

            ident = persist.tile([128, 128], BF, tag="ident")
            make_identity(nc, ident[:])
            ones = persist.tile([128, 128], BF, tag="ones")
            nc.vector.memset(ones[:], 1.0)
            tg_sb = persist.tile([128, H_LOC], F32, tag="tg_sb")
            nc.scalar.dma_start(tg_sb[:], tg.ap().partition_broadcast(128))
            akT_sb = persist.tile([128, H_LOC, AL], BF, tag="akT_sb")
            nc.scalar.dma_start(
                akT_sb[:], akT.ap().rearrange("p (h a) -> p h a", h=H_LOC))
            av_sb = persist.tile([AL, CH], BF, tag="av_sb")
            nc.scalar.dma_start(av_sb[:], av.ap())
            m01_sb = persist.tile([128, 4, 512], BF, tag="m01_sb")
            nc.scalar.dma_start(
                m01_sb[:], m01.ap().rearrange("p (j q) -> p j q", j=4))
            # RoPE tables in the baseline layout: [128 part, S//128, 64]
            cs_all = persist.tile([128, S // 128, HD // 2], F32, tag="cs_all")
            nc.scalar.dma_start(
                cs_all[:], fcos.ap().rearrange("(pb p) f -> p pb f", p=128))
            sn_all = persist.tile([128, S // 128, HD // 2], F32, tag="sn_all")
            nc.scalar.dma_start(
                sn_all[:], fsin.ap().rearrange("(pb p) f -> p pb f", p=128))

            # Phase B/C loads on the otherwise-idle gpsimd queue: they are
            # issued up front in its FIFO and fire as soon as the producing
            # spills' semaphores allow, prefetching across phase boundaries.
            def _bh_loads(b_i, h):
                qTb = at.tile([128, S], BF, tag="qTb", name="qTb")
                nc.gpsimd.dma_start(
                    qTb[:], qT_d[ds(h * HD, HD), ds(b_i * S, S)])
                kTb = at.tile([128, S], BF, tag="kTb", name="kTb")
                nc.gpsimd.dma_start(
                    kTb[:], kT_d[ds(h * HD, HD), ds(b_i * S, S)])
                vb2 = at.tile([128, S // 128, HD], BF, tag="vb2", name="vb2")
                nc.gpsimd.dma_start(
                    vb2[:],
                    v_d[ds(b_i * S, S), ts(h, HD)].rearrange(
                        "(kt p) d -> p kt d", p=128))
                return qTb, kTb, vb2

            # ================= phase A: QKV =================
            NPA = 16                  # 256-token panels
            with tc.tile_pool(name="wres", bufs=1) as wres, \
                 tc.tile_pool(name="xa", bufs=6) as xa, \
                 tc.tile_pool(name="ar", bufs=3) as ar, \
                 tc.tile_pool(name="aspill", bufs=2) as aspill, \
                 tc.tile_pool(name="ps_a", bufs=2, space="PSUM") as ps_a, \
                 tc.tile_pool(name="ps_t", bufs=2, space="PSUM") as ps_t:
                wTs = []
                for nm, wt in (("q", wqT), ("k", wkT), ("v", wvT)):
                    wT = wres.tile([128, NK, CH], BF, tag=f"wT{nm}",
                                   name=f"wT{nm}")
                    nc.scalar.dma_start(
                        wT[:], wt.ap().rearrange("(nk p) c -> p nk c", p=128))
                    wTs.append(wT)

                for pan in range(NPA):
                    # xT quarter-tiles for this panel: [128, 8, 256] each
                    xh = []
                    for qd in range(4):
                        xt = xa.tile([128, NK // 4, 256], BF, tag="xt")
                        nc.sync.dma_start(
                            xt[:],
                            xT.ap()[ds(qd * (D // 4), D // 4),
                                    ds(pan * 256, 256)].rearrange(
                                "(k p) t -> p k t", p=128))
                        xh.append(xt)
                    for ck in range(2):          # 128-token chunks in panel
                        tglob = pan * 2 + ck     # global token tile
                        srow = (tglob % (S // 128)) * 128  # within batch
                        csb = cs_all[:, srow // 128, :]
                        ssb = sn_all[:, srow // 128, :]
                        for p_i in range(3):     # chain-major: one PSUM bank
                            pp = ps_a.tile([128, CH], F32, tag=f"pp{p_i}",
                                           name=f"pp{p_i}")
                            for dt in range(NK):
                                lx = xh[dt // (NK // 4)][:, dt % (NK // 4),
                                                         ts(ck, 128)]
                                nc.tensor.matmul(
                                    pp[:], lhsT=lx, rhs=wTs[p_i][:, dt, :],
                                    start=(dt == 0), stop=(dt == NK - 1))
                            if p_i == 2:
                                # v: cast + spill natural
                                vb = ar.tile([128, CH], BF, tag="vb")
                                nc.scalar.activation(vb[:], pp[:], COPY)
                                nc.scalar.dma_start(
                                    v_d[ds(tglob * 128, 128), :], vb[:])
                                continue
                            # q, k: RoPE from PSUM -> natural bf16
                            dst = qT_d if p_i == 0 else kT_d
                            rp = ar.tile([128, CH], BF, tag=f"rp{p_i}",
                                         name=f"rp{p_i}")
                            for h in range(H_LOC):
                                pv2 = pp[:, ts(h, HD)].rearrange(
                                    "p (i two) -> p two i", two=2)
                                rv = rp[:, ts(h, HD)].rearrange(
                                    "p (i two) -> p two i", two=2)
                                a0, b0 = pv2[:, 0, :], pv2[:, 1, :]
                                t1 = ar.tile([128, HD // 2], F32, tag="t1")
                                t2 = ar.tile([128, HD // 2], F32, tag="t2")
                                nc.vector.tensor_mul(t1[:], a0, csb)
                                nc.vector.tensor_mul(t2[:], b0, ssb)
                                nc.vector.tensor_sub(rv[:, 0, :], t1[:], t2[:])
                                nc.vector.tensor_mul(t1[:], a0, ssb)
                                nc.vector.tensor_mul(t2[:], b0, csb)
                                nc.vector.tensor_add(rv[:, 1, :], t1[:], t2[:])
                            # transpose each 128-ch block to [ch, tok]
                            for ct in range(4):
                                tp = ps_t.tile([128, 128], BF, tag="tp")
                                nc.tensor.transpose(tp[:], rp[:, ts(ct, 128)],
                                                    ident[:])
                                st = aspill.tile([128, 128], BF, tag="st",
                                                 bufs=4)
                                nc.scalar.activation(st[:], tp[:], COPY)
                                nc.scalar.dma_start(
                                    dst[ds(ct * 128, 128),
                                        ds(tglob * 128, 128)], st[:])

            # ================= phase B: attention =================
            obc_cm = tc.tile_pool(name="obc", bufs=1)
            obc = obc_cm.__enter__()
            woT_sb = obc.tile([128, H_LOC, D], BF, tag="woT_sb")
            nc.scalar.dma_start(
                woT_sb[:], woT.ap().rearrange("(c p) d -> p c d", p=128))
            oT_sb = obc.tile([128, H_LOC, TOK], BF, tag="oT_sb")
            with tc.tile_pool(name="att", bufs=3) as att, \
                 tc.tile_pool(name="ps_st", bufs=4, space="PSUM") as ps_st, \
                 tc.tile_pool(name="ps_ac", bufs=1, space="PSUM") as ps_ac:
                cur = _bh_loads(0, 0)
                for bh in range(B * H_LOC):
                    b_i, h = divmod(bh, H_LOC)
                    nxt = (_bh_loads(*divmod(bh + 1, H_LOC))
                           if bh + 1 < B * H_LOC else None)
                    qTb, kTb, vb2 = cur
                    for qc in range(NQC):
                        nkt = (qc + 1) * 4
                        stb = att.tile([128, S // 128, 512], BF, tag="stb",
                                       bufs=2)
                        for kt in range(nkt):
                            sps = ps_st.tile([128, 512], F32, tag="sps")
                            nc.tensor.matmul(sps[:],
                                             lhsT=kTb[:, ts(kt, 128)],
                                             rhs=qTb[:, ts(qc, 512)],
                                             start=True, stop=True)
                            nc.scalar.activation(stb[:, kt, :], sps[:],
                                                 EXP, scale=SCALE)
                            if kt // 4 == qc:
                                # diagonal block: multiplicative 0/1 mask
                                nc.vector.tensor_mul(
                                    stb[:, kt, :], stb[:, kt, :],
                                    m01_sb[:, kt % 4, :])
                        # adapter scores [AL, 512]
                        spa = ps_st.tile([128, 512], F32, tag="sps")
                        nc.tensor.matmul(spa[:AL, :], lhsT=akT_sb[:, h, :],
                                         rhs=qTb[:, ts(qc, 512)],
                                         start=True, stop=True)
                        pab = att.tile([AL, 512], BF, tag="pab")
                        nc.scalar.activation(pab[:], spa[:AL, :], EXP,
                                             scale=SCALE)
                        # 4:1 DVE pre-reduction, then column sums via matmul
                        nred = nkt // 4
                        sadd = att.tile([128, NQC, 512], BF, tag="sadd",
                                        bufs=2)
                        for g in range(nred):
                            nc.vector.tensor_add(sadd[:, g, :],
                                                 stb[:, 4 * g, :],
                                                 stb[:, 4 * g + 1, :])
                            nc.vector.tensor_add(sadd[:, g, :],
                                                 sadd[:, g, :],
                                                 stb[:, 4 * g + 2, :])
                            nc.vector.tensor_add(sadd[:, g, :],
                                                 sadd[:, g, :],
                                                 stb[:, 4 * g + 3, :])
                        s2 = ps_ac.tile([33, 512], F32, tag="s2")
                        for g in range(nred):
                            nc.tensor.matmul(s2[0:1, :], lhsT=ones[:, 0:1],
                                             rhs=sadd[:, g, :],
                                             start=(g == 0),
                                             stop=(g == nred - 1))
                        nc.tensor.matmul(s2[32:33, :], lhsT=ones[:AL, 0:1],
                                         rhs=pab[:], start=True, stop=True)
                        # PV accumulation: oT [128 d, 512 q]
                        o_ps = ps_ac.tile([128, 512], F32, tag="o_ps", bufs=2)
                        for kt in range(nkt):
                            nc.tensor.matmul(o_ps[:], lhsT=vb2[:, kt, :],
                                             rhs=stb[:, kt, :],
                                             start=(kt == 0),
                                             stop=(kt == nkt - 1))
                        oa_ps = ps_ac.tile([128, 512], F32, tag="oa_ps")
                        nc.tensor.matmul(oa_ps[:], lhsT=av_sb[:, ts(h, HD)],
                                         rhs=pab[:], start=True, stop=True)
                        # combine: o = o_main/s_main + tanh(g)*oa/s_adapt
                        sb2 = att.tile([33, 512], BF, tag="sb2")
                        nc.vector.tensor_copy(sb2[0:1, :], s2[0:1, :])
                        nc.vector.tensor_copy(sb2[32:33, :], s2[32:33, :])
                        bc_ps = ps_st.tile([128, 512], F32, tag="sps")
                        nc.tensor.matmul(bc_ps[:], lhsT=ones[0:1, :],
                                         rhs=sb2[0:1, :], start=True,
                                         stop=True)
                        bca_ps = ps_st.tile([128, 512], F32, tag="sps")
                        nc.tensor.matmul(bca_ps[:], lhsT=ones[32:33, :],
                                         rhs=sb2[32:33, :], start=True,
                                         stop=True)
                        rb = att.tile([128, 512], F32, tag="rb")
                        nc.vector.reciprocal_approx_fast(rb[:], bc_ps[:])
                        rba = att.tile([128, 512], F32, tag="rba")
                        nc.vector.reciprocal_approx_fast(rba[:], bca_ps[:])
                        t3 = att.tile([128, 512], F32, tag="t3")
                        nc.vector.tensor_mul(t3[:], o_ps[:], rb[:])
                        t4 = att.tile([128, 512], F32, tag="t4")
                        nc.vector.scalar_tensor_tensor(
                            t4[:], rba[:], tg_sb[:, ds(h, 1)], oa_ps[:],
                            op0=MULT, op1=MULT)
                        nc.vector.tensor_add(
                            oT_sb[:, h, ds(b_i * S + qc * 512, 512)],
                            t3[:], t4[:])
                    cur = nxt

            # ======= phase C: RowParallel wo (y^T) + ReduceScatter =======
            with tc.tile_pool(name="wy", bufs=4) as wy, \
                 tc.tile_pool(name="ps_y", bufs=8, space="PSUM") as ps_y:
                for dq in range(NDQ):
                    for dtile in range(DQW // 128):
                        drow = dq * DQW + dtile * 128
                        yts = [ps_y.tile([128, 512], F32, tag=f"yt{i}",
                                         name=f"yt{i}", bufs=2)
                               for i in range(4)]
                        for tch in range(2):     # token halves
                            for ct in range(H_LOC):
                                for tc4 in range(4):
                                    tcg = tch * 4 + tc4
                                    nc.tensor.matmul(
                                        yts[tc4][:],
                                        lhsT=woT_sb[:, ct, ds(drow, 128)],
                                        rhs=oT_sb[:, ct, ts(tcg, 512)],
                                        start=(ct == 0),
                                        stop=(ct == H_LOC - 1))
                            for tc4 in range(4):
                                tcg = tch * 4 + tc4
                                yb = wy.tile([128, 512], BF, tag="yb")
                                nc.scalar.activation(yb[:], yts[tc4][:], COPY)
                                nc.scalar.dma_start(
                                    yT_ds[dq][ds(dtile * 128, 128),
                                              ts(tcg, 512)], yb[:])
                            if tch == 0:
                                yts = [ps_y.tile([128, 512], F32,
                                                 tag=f"yt{i}", name=f"yt{i}",
                                                 bufs=2) for i in range(4)]
                    nc.gpsimd.collective_compute(
                        "ReduceScatter", mybir.AluOpType.add,
                        replica_groups=[list(range(N_CORES))],
                        ins=[yT_ds[dq].opt()], outs=[yr_ds[dq].opt()])
                # reduced shards -> f32 output via SWDGE cast-DMA
                for dq in range(NDQ):
                    nc.gpsimd.dma_start(
                        out.ap()[ds(dq * (DQW // N_CORES), DQW // N_CORES),
                                 :],
                        yr_ds[dq][:, :])
            obc_cm.__exit__(None, None, None)
    nc.compile()
    return nc


_NC_CACHE = None


def _prep(x, wq, wk, wv, wo, gate, adapter, freqs_cos, freqs_sin, mask):
    """Host-side layout prep. Returns per-core input maps."""
    xf = np.asarray(x, np.float32).reshape(TOK, D)
    xT = np.ascontiguousarray(xf.T).astype(BF_NP)
    wq = np.asarray(wq, np.float32)
    wk = np.asarray(wk, np.float32)
    wv = np.asarray(wv, np.float32)
    wo = np.asarray(wo, np.float32)
    g = np.tanh(np.asarray(gate, np.float32).reshape(H))
    ad = np.asarray(adapter, np.float32).reshape(AL, D)
    a_k = ad @ wk.T          # [AL, H*HD]
    a_v = ad @ wv.T
    fc = np.ascontiguousarray(np.asarray(freqs_cos, np.float32))
    fs = np.ascontiguousarray(np.asarray(freqs_sin, np.float32))
    mk = np.asarray(mask, np.float32).reshape(S, S)
    # multiplicative 0/1 diagonal masks, S^T orientation: m01[j][k, q]
    m01 = np.empty((128, 4, 512), np.float32)
    for j in range(4):
        blk = mk[0:512, j * 128:(j + 1) * 128]    # [q, k] additive
        m01[:, j, :] = (blk == 0.0).T.astype(np.float32)
    m01 = np.ascontiguousarray(m01.reshape(128, 4 * 512)).astype(BF_NP)

    in_maps = []
    for r in range(N_CORES):
        sl = slice(r * CH, (r + 1) * CH)
        akr = a_k[:, sl]     # [AL, CH]
        akT = np.zeros((HD, H_LOC, AL), np.float32)
        for h in range(H_LOC):
            akT[:, h, :] = akr[:, h * HD:(h + 1) * HD].T
        in_maps.append({
            "xT": xT,
            "wqT": np.ascontiguousarray(wq[sl].T).astype(BF_NP),
            "wkT": np.ascontiguousarray(wk[sl].T).astype(BF_NP),
            "wvT": np.ascontiguousarray(wv[sl].T).astype(BF_NP),
            "woT": np.ascontiguousarray(wo[:, sl].T).astype(BF_NP),
            "fcos": fc,
            "fsin": fs,
            "akT": np.ascontiguousarray(
                akT.reshape(HD, H_LOC * AL)).astype(BF_NP),
            "av": np.ascontiguousarray(a_v[:, sl]).astype(BF_NP),
            "tg": np.ascontiguousarray(
                g[r * H_LOC:(r + 1) * H_LOC].reshape(1, H_LOC)),
            "m01": m01,
        })
    return in_maps


def kernel(x, wq, wk, wv, wo, gate, adapter, freqs_cos, freqs_sin, mask,
           start_pos=0, **_unused):
    global _NC_CACHE
    if _NC_CACHE is None:
        _NC_CACHE = build()
    nc = _NC_CACHE
    in_maps = _prep(x, wq, wk, wv, wo, gate, adapter,
                    freqs_cos, freqs_sin, mask)
    res = bass_utils.run_bass_kernel_spmd(nc, in_maps,
                                          core_ids=list(range(N_CORES)))
    # out[r] rows (dq*128+p) hold summed yT rows dq*1024 + r*128 + p
    yT = np.empty((TOK, TOK), np.float32)
    for r in range(N_CORES):
        arr = np.asarray(res.results[r]["out"]).reshape(NDQ, 128, TOK)
        for dq in range(NDQ):
            yT[dq * DQW + r * 128: dq * DQW + r * 128 + 128, :] = arr[dq]
    return np.ascontiguousarray(yT.T).reshape(B, S, D)


if __name__ == "__main__":
    nc = build()
    print("compiled ok, instrs:",
          sum(len(bb.instructions) for f in nc.m.functions for bb in f.blocks))


# revision 14
# speedup vs baseline: 1.3951x; 1.1636x over previous
"""Distributed Trainium2 kernel for the gated-adapter attention module.

Head-parallel tensor parallelism over 8 NeuronCores (4 heads each).
Host-side prep (inside kernel()): inputs are pre-transposed and
pre-cast to bf16 so the device never transposes weights or x —
xT [D, TOK], wqT/wkT/wvT [D, CH], and the core's wo column-slice
woT [CH, D] arrive matmul-ready.  The adapter K/V projections
(10x4096 @ 4096x512) and tanh(gate) are precomputed on host.

Device pipeline per core:
  A) QKV: x-stationary matmuls emit q/k/v natural [tok, ch]; RoPE on
     DVE straight out of PSUM; q/k PE-transposed to [ch, tok] and
     spilled to DRAM (contiguous loads later), v spilled natural.
  B) Attention per (batch, head) in S^T orientation: scores [k, q],
     uniform exp on ACT, multiplicative 0/1 diagonal masks on DVE,
     4:1 DVE pre-reduction then softmax sums via ones-matmul, PV
     accumulation, gated adapter branch; o^T accumulates in SBUF.
  C) RowParallel wo in y^T orientation: yT[dout, tok] = woT.T @ o^T
     with woT stationary (8 PSUM banks per dout-tile), partials
     ReduceScattered over dout in 4 chunks (pipelined against the
     matmuls); final f32 via SWDGE cast-DMA; host re-transposes.

Engine map: sync = x prefetch only; scalar = weights + compute-paced
copies/spills/exp; gpsimd = phase-B loads + RS triggers + output
(fire on data semaphores, never queue behind compute).
"""

import sys

sys.path.insert(0, "/opt/trn_rl_repo")

import numpy as np
import ml_dtypes

import concourse.bass as bass
import concourse.mybir as mybir
import concourse.tile as tile
from concourse import bacc, bass_utils
from concourse.bass import ds, ts
from concourse.masks import make_identity

N_CORES = 8
B, S, D = 2, 2048, 4096
H = 32
HD = 128                      # head dim
H_LOC = H // N_CORES          # 4 heads per core
CH = H_LOC * HD               # 512 local channels
TOK = B * S                   # 4096 tokens
NK = D // 128                 # 32 contraction tiles
AL = 10                       # adapter length
NQC = S // 512                # 4 query chunks per sequence
NDQ = 4                       # dout chunks for the wo/RS pipeline
DQW = D // NDQ                # 1024 dout rows per chunk
SCALE = 1.0 / float(np.sqrt(HD))
BF = mybir.dt.bfloat16
F32 = mybir.dt.float32
EXP = mybir.ActivationFunctionType.Exp
COPY = mybir.ActivationFunctionType.Copy
MULT = mybir.AluOpType.mult
BF_NP = ml_dtypes.bfloat16


def build():
    nc = bacc.Bacc("TRN2", target_bir_lowering=False, debug=False,
                   num_devices=N_CORES)
    xT = nc.dram_tensor("xT", [D, TOK], BF, kind="ExternalInput")
    wqT = nc.dram_tensor("wqT", [D, CH], BF, kind="ExternalInput")
    wkT = nc.dram_tensor("wkT", [D, CH], BF, kind="ExternalInput")
    wvT = nc.dram_tensor("wvT", [D, CH], BF, kind="ExternalInput")
    woTF = nc.dram_tensor("woTF", [D, D], BF, kind="ExternalInput")
    fcos = nc.dram_tensor("fcos", [S, HD // 2], F32, kind="ExternalInput")
    fsin = nc.dram_tensor("fsin", [S, HD // 2], F32, kind="ExternalInput")
    akT = nc.dram_tensor("akT", [HD, H_LOC * AL], BF, kind="ExternalInput")
    av = nc.dram_tensor("av", [AL, CH], BF, kind="ExternalInput")
    tg = nc.dram_tensor("tg", [1, H_LOC], F32, kind="ExternalInput")
    m01 = nc.dram_tensor("m01", [128, 4 * 512], BF, kind="ExternalInput")
    out = nc.dram_tensor("out", [TOK // N_CORES, D], F32,
                         kind="ExternalOutput")

    with tile.TileContext(nc) as tc:
        with tc.tile_pool(name="dram", bufs=1, space="DRAM") as dram, \
             tc.tile_pool(name="persist", bufs=1) as persist:
            at_cm = tc.tile_pool(name="at", bufs=3)
            at = at_cm.__enter__()
            qT_d = dram.tile([CH, TOK], BF, tag="qT_d")
            kT_d = dram.tile([CH, TOK], BF, tag="kT_d")
            v_d = dram.tile([TOK, CH], BF, tag="v_d")
            a2a_in = dram.tile([N_CORES, CH, TOK // N_CORES], BF,
                               tag="a2a_in")
            a2a_out = dram.tile([N_CORES, CH, TOK // N_CORES], BF,
                                tag="a2a_out")

            ident = persist.tile([128, 128], BF, tag="ident")
            make_identity(nc, ident[:])
            ones = persist.tile([128, 128], BF, tag="ones")
            nc.vector.memset(ones[:], 1.0)
            tg_sb = persist.tile([128, H_LOC], F32, tag="tg_sb")
            nc.scalar.dma_start(tg_sb[:], tg.ap().partition_broadcast(128))
            akT_sb = persist.tile([128, H_LOC, AL], BF, tag="akT_sb")
            nc.scalar.dma_start(
                akT_sb[:], akT.ap().rearrange("p (h a) -> p h a", h=H_LOC))
            av_sb = persist.tile([AL, CH], BF, tag="av_sb")
            nc.scalar.dma_start(av_sb[:], av.ap())
            m01_sb = persist.tile([128, 4, 512], BF, tag="m01_sb")
            nc.scalar.dma_start(
                m01_sb[:], m01.ap().rearrange("p (j q) -> p j q", j=4))
            # RoPE tables in the baseline layout: [128 part, S//128, 64]
            cs_all = persist.tile([128, S // 128, HD // 2], F32, tag="cs_all")
            nc.scalar.dma_start(
                cs_all[:], fcos.ap().rearrange("(pb p) f -> p pb f", p=128))
            sn_all = persist.tile([128, S // 128, HD // 2], F32, tag="sn_all")
            nc.scalar.dma_start(
                sn_all[:], fsin.ap().rearrange("(pb p) f -> p pb f", p=128))

            # Phase B/C loads on the otherwise-idle gpsimd queue: they are
            # issued up front in its FIFO and fire as soon as the producing
            # spills' semaphores allow, prefetching across phase boundaries.
            def _bh_loads(b_i, h):
                qTb = at.tile([128, S], BF, tag="qTb", name="qTb")
                nc.gpsimd.dma_start(
                    qTb[:], qT_d[ds(h * HD, HD), ds(b_i * S, S)])
                kTb = at.tile([128, S], BF, tag="kTb", name="kTb")
                nc.gpsimd.dma_start(
                    kTb[:], kT_d[ds(h * HD, HD), ds(b_i * S, S)])
                vb2 = at.tile([128, S // 128, HD], BF, tag="vb2", name="vb2")
                nc.gpsimd.dma_start(
                    vb2[:],
                    v_d[ds(b_i * S, S), ts(h, HD)].rearrange(
                        "(kt p) d -> p kt d", p=128))
                return qTb, kTb, vb2

            # ================= phase A: QKV =================
            NPA = 16                  # 256-token panels
            with tc.tile_pool(name="wres", bufs=1) as wres, \
                 tc.tile_pool(name="xa", bufs=6) as xa, \
                 tc.tile_pool(name="ar", bufs=3) as ar, \
                 tc.tile_pool(name="aspill", bufs=2) as aspill, \
                 tc.tile_pool(name="ps_a", bufs=2, space="PSUM") as ps_a, \
                 tc.tile_pool(name="ps_t", bufs=2, space="PSUM") as ps_t:
                wTs = []
                for nm, wt in (("q", wqT), ("k", wkT), ("v", wvT)):
                    wT = wres.tile([128, NK, CH], BF, tag=f"wT{nm}",
                                   name=f"wT{nm}")
                    nc.scalar.dma_start(
                        wT[:], wt.ap().rearrange("(nk p) c -> p nk c", p=128))
                    wTs.append(wT)

                for pan in range(NPA):
                    # xT quarter-tiles for this panel: [128, 8, 256] each
                    xh = []
                    for qd in range(4):
                        xt = xa.tile([128, NK // 4, 256], BF, tag="xt")
                        nc.sync.dma_start(
                            xt[:],
                            xT.ap()[ds(qd * (D // 4), D // 4),
                                    ds(pan * 256, 256)].rearrange(
                                "(k p) t -> p k t", p=128))
                        xh.append(xt)
                    for ck in range(2):          # 128-token chunks in panel
                        tglob = pan * 2 + ck     # global token tile
                        srow = (tglob % (S // 128)) * 128  # within batch
                        csb = cs_all[:, srow // 128, :]
                        ssb = sn_all[:, srow // 128, :]
                        for p_i in range(3):     # chain-major: one PSUM bank
                            pp = ps_a.tile([128, CH], F32, tag=f"pp{p_i}",
                                           name=f"pp{p_i}")
                            for dt in range(NK):
                                lx = xh[dt // (NK // 4)][:, dt % (NK // 4),
                                                         ts(ck, 128)]
                                nc.tensor.matmul(
                                    pp[:], lhsT=lx, rhs=wTs[p_i][:, dt, :],
                                    start=(dt == 0), stop=(dt == NK - 1))
                            if p_i == 2:
                                # v: cast + spill natural
                                vb = ar.tile([128, CH], BF, tag="vb")
                                nc.scalar.activation(vb[:], pp[:], COPY)
                                nc.scalar.dma_start(
                                    v_d[ds(tglob * 128, 128), :], vb[:])
                                continue
                            # q, k: RoPE from PSUM -> natural bf16
                            dst = qT_d if p_i == 0 else kT_d
                            rp = ar.tile([128, CH], BF, tag=f"rp{p_i}",
                                         name=f"rp{p_i}")
                            for h in range(H_LOC):
                                pv2 = pp[:, ts(h, HD)].rearrange(
                                    "p (i two) -> p two i", two=2)
                                rv = rp[:, ts(h, HD)].rearrange(
                                    "p (i two) -> p two i", two=2)
                                a0, b0 = pv2[:, 0, :], pv2[:, 1, :]
                                t1 = ar.tile([128, HD // 2], F32, tag="t1")
                                t2 = ar.tile([128, HD // 2], F32, tag="t2")
                                nc.vector.tensor_mul(t1[:], a0, csb)
                                nc.vector.tensor_mul(t2[:], b0, ssb)
                                nc.vector.tensor_sub(rv[:, 0, :], t1[:], t2[:])
                                nc.vector.tensor_mul(t1[:], a0, ssb)
                                nc.vector.tensor_mul(t2[:], b0, csb)
                                nc.vector.tensor_add(rv[:, 1, :], t1[:], t2[:])
                            # transpose each 128-ch block to [ch, tok]
                            for ct in range(4):
                                tp = ps_t.tile([128, 128], BF, tag="tp")
                                nc.tensor.transpose(tp[:], rp[:, ts(ct, 128)],
                                                    ident[:])
                                st = aspill.tile([128, 128], BF, tag="st",
                                                 bufs=4)
                                nc.scalar.activation(st[:], tp[:], COPY)
                                nc.scalar.dma_start(
                                    dst[ds(ct * 128, 128),
                                        ds(tglob * 128, 128)], st[:])

            # ================= phase B: attention =================
            with tc.tile_pool(name="att", bufs=3) as att, \
                 tc.tile_pool(name="ps_st", bufs=4, space="PSUM") as ps_st, \
                 tc.tile_pool(name="ps_ac", bufs=1, space="PSUM") as ps_ac:
                cur = _bh_loads(0, 0)
                for bh in range(B * H_LOC):
                    b_i, h = divmod(bh, H_LOC)
                    nxt = (_bh_loads(*divmod(bh + 1, H_LOC))
                           if bh + 1 < B * H_LOC else None)
                    qTb, kTb, vb2 = cur
                    for qc in range(NQC):
                        nkt = (qc + 1) * 4
                        stb = att.tile([128, S // 128, 512], BF, tag="stb",
                                       bufs=2)
                        sadd = att.tile([128, NQC, 512], BF, tag="sadd",
                                        bufs=2)
                        s2 = ps_ac.tile([33, 512], F32, tag="s2")
                        o_ps = ps_ac.tile([128, 512], F32, tag="o_ps", bufs=2)

                        def _score(kt):
                            sps = ps_st.tile([128, 512], F32, tag="sps")
                            nc.tensor.matmul(sps[:],
                                             lhsT=kTb[:, ts(kt, 128)],
                                             rhs=qTb[:, ts(qc, 512)],
                                             start=True, stop=True)
                            nc.scalar.activation(stb[:, kt, :], sps[:],
                                                 EXP, scale=SCALE)
                            if kt // 4 == qc:
                                nc.vector.tensor_mul(
                                    stb[:, kt, :], stb[:, kt, :],
                                    m01_sb[:, kt % 4, :])

                        def _pv(kt):
                            nc.tensor.matmul(o_ps[:], lhsT=vb2[:, kt, :],
                                             rhs=stb[:, kt, :],
                                             start=(kt == 0),
                                             stop=(kt == nkt - 1))
                            if kt % 4 == 3:
                                g = kt // 4
                                nc.vector.tensor_add(sadd[:, g, :],
                                                     stb[:, 4 * g, :],
                                                     stb[:, 4 * g + 1, :])
                                nc.vector.tensor_add(sadd[:, g, :],
                                                     sadd[:, g, :],
                                                     stb[:, 4 * g + 2, :])
                                nc.vector.tensor_add(sadd[:, g, :],
                                                     sadd[:, g, :],
                                                     stb[:, 4 * g + 3, :])
                                nc.tensor.matmul(s2[0:1, :],
                                                 lhsT=ones[:, 0:1],
                                                 rhs=sadd[:, g, :],
                                                 start=(g == 0),
                                                 stop=(g == nkt // 4 - 1))

                        _score(0)
                        for kt in range(1, nkt):
                            _score(kt)
                            _pv(kt - 1)
                        _pv(nkt - 1)
                        # adapter scores [AL, 512]
                        spa = ps_st.tile([128, 512], F32, tag="sps")
                        nc.tensor.matmul(spa[:AL, :], lhsT=akT_sb[:, h, :],
                                         rhs=qTb[:, ts(qc, 512)],
                                         start=True, stop=True)
                        pab = att.tile([AL, 512], BF, tag="pab")
                        nc.scalar.activation(pab[:], spa[:AL, :], EXP,
                                             scale=SCALE)
                        nc.tensor.matmul(s2[32:33, :], lhsT=ones[:AL, 0:1],
                                         rhs=pab[:], start=True, stop=True)
                        oa_ps = ps_ac.tile([128, 512], F32, tag="oa_ps")
                        nc.tensor.matmul(oa_ps[:], lhsT=av_sb[:, ts(h, HD)],
                                         rhs=pab[:], start=True, stop=True)
                        # combine: o = o_main/s_main + tanh(g)*oa/s_adapt
                        sb2 = att.tile([33, 512], BF, tag="sb2")
                        nc.vector.tensor_copy(sb2[0:1, :], s2[0:1, :])
                        nc.vector.tensor_copy(sb2[32:33, :], s2[32:33, :])
                        bc_ps = ps_st.tile([128, 512], F32, tag="sps")
                        nc.tensor.matmul(bc_ps[:], lhsT=ones[0:1, :],
                                         rhs=sb2[0:1, :], start=True,
                                         stop=True)
                        bca_ps = ps_st.tile([128, 512], F32, tag="sps")
                        nc.tensor.matmul(bca_ps[:], lhsT=ones[32:33, :],
                                         rhs=sb2[32:33, :], start=True,
                                         stop=True)
                        rb = att.tile([128, 512], F32, tag="rb")
                        nc.vector.reciprocal_approx_fast(rb[:], bc_ps[:])
                        rba = att.tile([128, 512], F32, tag="rba")
                        nc.vector.reciprocal_approx_fast(rba[:], bca_ps[:])
                        t3 = att.tile([128, 512], F32, tag="t3")
                        nc.vector.tensor_mul(t3[:], o_ps[:], rb[:])
                        t4 = att.tile([128, 512], F32, tag="t4")
                        nc.vector.scalar_tensor_tensor(
                            t4[:], rba[:], tg_sb[:, ds(h, 1)], oa_ps[:],
                            op0=MULT, op1=MULT)
                        ob = att.tile([128, 512], BF, tag="ob")
                        nc.vector.tensor_add(ob[:], t3[:], t4[:])
                        nc.scalar.dma_start(
                            a2a_in[b_i * NQC + qc][ds(h * HD, HD), :], ob[:])
                    cur = nxt

            at_cm.__exit__(None, None, None)
            # ========== phase C: AllToAll + full-wo quarters ==========
            nc.gpsimd.collective_compute(
                "AllToAll", mybir.AluOpType.bypass,
                replica_groups=[list(range(N_CORES))],
                ins=[a2a_in.opt()], outs=[a2a_out.opt()])
            with tc.tile_pool(name="wof", bufs=2) as wof, \
                 tc.tile_pool(name="wy", bufs=4) as wy, \
                 tc.tile_pool(name="ps_y", bufs=2, space="PSUM") as ps_y:
                oTf = wof.tile([128, NK, TOK // N_CORES], BF, tag="oTf",
                               bufs=1)
                for sc in range(N_CORES):
                    nc.gpsimd.dma_start(
                        oTf[:, ds(sc * H_LOC, H_LOC), :],
                        a2a_out[sc].rearrange("(c p) t -> p c t", p=128))
                for dq in range(NDQ):
                    wqt = wof.tile([128, NK, DQW], BF, tag="wqt")
                    nc.sync.dma_start(
                        wqt[:],
                        woTF.ap()[:, ds(dq * DQW, DQW)].rearrange(
                            "(ct p) d -> p ct d", p=128))
                    for tt in range(4):
                        yt = ps_y.tile([128, DQW], F32, tag="yt")
                        for ct in range(NK):
                            for dc in range(DQW // 512):
                                nc.tensor.matmul(
                                    yt[:, ts(dc, 512)],
                                    lhsT=oTf[:, ct, ts(tt, 128)],
                                    rhs=wqt[:, ct, ts(dc, 512)],
                                    start=(ct == 0), stop=(ct == NK - 1))
                        yf = wy.tile([128, DQW], F32, tag="yf")
                        nc.scalar.activation(yf[:], yt[:], COPY)
                        nc.scalar.dma_start(
                            out.ap()[ds(tt * 128, 128), ds(dq * DQW, DQW)],
                            yf[:])
    nc.compile()
    return nc


_NC_CACHE = None


def _prep(x, wq, wk, wv, wo, gate, adapter, freqs_cos, freqs_sin, mask):
    """Host-side layout prep. Returns per-core input maps."""
    xf = np.asarray(x, np.float32).reshape(TOK, D)
    xT = np.ascontiguousarray(xf.T).astype(BF_NP)
    wq = np.asarray(wq, np.float32)
    wk = np.asarray(wk, np.float32)
    wv = np.asarray(wv, np.float32)
    wo = np.asarray(wo, np.float32)
    g = np.tanh(np.asarray(gate, np.float32).reshape(H))
    ad = np.asarray(adapter, np.float32).reshape(AL, D)
    a_k = ad @ wk.T          # [AL, H*HD]
    a_v = ad @ wv.T
    fc = np.ascontiguousarray(np.asarray(freqs_cos, np.float32))
    fs = np.ascontiguousarray(np.asarray(freqs_sin, np.float32))
    woTF = np.ascontiguousarray(wo.T).astype(BF_NP)
    mk = np.asarray(mask, np.float32).reshape(S, S)
    # multiplicative 0/1 diagonal masks, S^T orientation: m01[j][k, q]
    m01 = np.empty((128, 4, 512), np.float32)
    for j in range(4):
        blk = mk[0:512, j * 128:(j + 1) * 128]    # [q, k] additive
        m01[:, j, :] = (blk == 0.0).T.astype(np.float32)
    m01 = np.ascontiguousarray(m01.reshape(128, 4 * 512)).astype(BF_NP)

    in_maps = []
    for r in range(N_CORES):
        sl = slice(r * CH, (r + 1) * CH)
        akr = a_k[:, sl]     # [AL, CH]
        akT = np.zeros((HD, H_LOC, AL), np.float32)
        for h in range(H_LOC):
            akT[:, h, :] = akr[:, h * HD:(h + 1) * HD].T
        in_maps.append({
            "xT": xT,
            "wqT": np.ascontiguousarray(wq[sl].T).astype(BF_NP),
            "wkT": np.ascontiguousarray(wk[sl].T).astype(BF_NP),
            "wvT": np.ascontiguousarray(wv[sl].T).astype(BF_NP),
            "woTF": woTF,
            "fcos": fc,
            "fsin": fs,
            "akT": np.ascontiguousarray(
                akT.reshape(HD, H_LOC * AL)).astype(BF_NP),
            "av": np.ascontiguousarray(a_v[:, sl]).astype(BF_NP),
            "tg": np.ascontiguousarray(
                g[r * H_LOC:(r + 1) * H_LOC].reshape(1, H_LOC)),
            "m01": m01,
        })
    return in_maps


def kernel(x, wq, wk, wv, wo, gate, adapter, freqs_cos, freqs_sin, mask,
           start_pos=0, **_unused):
    global _NC_CACHE
    if _NC_CACHE is None:
        _NC_CACHE = build()
    nc = _NC_CACHE
    in_maps = _prep(x, wq, wk, wv, wo, gate, adapter,
                    freqs_cos, freqs_sin, mask)
    res = bass_utils.run_bass_kernel_spmd(nc, in_maps,
                                          core_ids=list(range(N_CORES)))
    y = np.concatenate([res.results[r]["out"] for r in range(N_CORES)], axis=0)
    return y.reshape(B, S, D)


if __name__ == "__main__":
    nc = build()
    print("compiled ok, instrs:",
          sum(len(bb.instructions) for f in nc.m.functions for bb in f.blocks))


# revision 23
# speedup vs baseline: 1.4754x; 1.0575x over previous
"""Distributed Trainium2 kernel for the gated-adapter attention module.

Head-parallel tensor parallelism over 8 NeuronCores (4 heads each).
Host-side prep (inside kernel()): inputs are pre-transposed and
pre-cast to bf16 so the device never transposes weights or x —
xT [D, TOK], wqT/wkT/wvT [D, CH], and the core's wo column-slice
woT [CH, D] arrive matmul-ready.  The adapter K/V projections
(10x4096 @ 4096x512) and tanh(gate) are precomputed on host.

Device pipeline per core:
  A) QKV: x-stationary matmuls emit q/k/v natural [tok, ch]; RoPE on
     DVE straight out of PSUM; q/k PE-transposed to [ch, tok] and
     spilled to DRAM (contiguous loads later), v spilled natural.
  B) Attention per (batch, head) in S^T orientation: scores [k, q],
     uniform exp on ACT, multiplicative 0/1 diagonal masks on DVE,
     4:1 DVE pre-reduction then softmax sums via ones-matmul, PV
     accumulation, gated adapter branch; o^T accumulates in SBUF.
  C) RowParallel wo in y^T orientation: yT[dout, tok] = woT.T @ o^T
     with woT stationary (8 PSUM banks per dout-tile), partials
     ReduceScattered over dout in 4 chunks (pipelined against the
     matmuls); final f32 via SWDGE cast-DMA; host re-transposes.

Engine map: sync = x prefetch only; scalar = weights + compute-paced
copies/spills/exp; gpsimd = phase-B loads + RS triggers + output
(fire on data semaphores, never queue behind compute).
"""

import sys

sys.path.insert(0, "/opt/trn_rl_repo")

import numpy as np
import ml_dtypes

import concourse.bass as bass
import concourse.mybir as mybir
import concourse.tile as tile
from concourse import bacc, bass_utils
from concourse.bass import ds, ts
from concourse.masks import make_identity

N_CORES = 8
B, S, D = 2, 2048, 4096
H = 32
HD = 128                      # head dim
H_LOC = H // N_CORES          # 4 heads per core
CH = H_LOC * HD               # 512 local channels
TOK = B * S                   # 4096 tokens
NK = D // 128                 # 32 contraction tiles
AL = 10                       # adapter length
NQC = S // 512                # 4 query chunks per sequence
NDQ = 4                       # dout chunks for the wo/RS pipeline
DQW = D // NDQ                # 1024 dout rows per chunk
SCALE = 1.0 / float(np.sqrt(HD))
BF = mybir.dt.bfloat16
F32 = mybir.dt.float32
EXP = mybir.ActivationFunctionType.Exp
COPY = mybir.ActivationFunctionType.Copy
MULT = mybir.AluOpType.mult
BF_NP = ml_dtypes.bfloat16


def build():
    nc = bacc.Bacc("TRN2", target_bir_lowering=False, debug=False,
                   num_devices=N_CORES)
    xT = nc.dram_tensor("xT", [D, TOK], BF, kind="ExternalInput")
    wqT = nc.dram_tensor("wqT", [D, CH], BF, kind="ExternalInput")
    wkT = nc.dram_tensor("wkT", [D, CH], BF, kind="ExternalInput")
    wvT = nc.dram_tensor("wvT", [D, CH], BF, kind="ExternalInput")
    woTF = nc.dram_tensor("woTF", [D, D], BF, kind="ExternalInput")
    fcos = nc.dram_tensor("fcos", [S, HD // 2], F32, kind="ExternalInput")
    fsin = nc.dram_tensor("fsin", [S, HD // 2], F32, kind="ExternalInput")
    akT = nc.dram_tensor("akT", [HD, H_LOC * AL], BF, kind="ExternalInput")
    av = nc.dram_tensor("av", [AL, CH], BF, kind="ExternalInput")
    tg = nc.dram_tensor("tg", [1, H_LOC], F32, kind="ExternalInput")
    m01 = nc.dram_tensor("m01", [128, 4 * 512], BF, kind="ExternalInput")
    out = nc.dram_tensor("out", [TOK // N_CORES, D], F32,
                         kind="ExternalOutput")

    with tile.TileContext(nc) as tc:
        with tc.tile_pool(name="dram", bufs=1, space="DRAM") as dram, \
             tc.tile_pool(name="persist", bufs=1) as persist:
            at_cm = tc.tile_pool(name="at", bufs=3)
            at = at_cm.__enter__()
            qT_d = dram.tile([CH, TOK], BF, tag="qT_d")
            kT_d = dram.tile([CH, TOK], BF, tag="kT_d")
            v_d = dram.tile([TOK, CH], BF, tag="v_d")
            a2a_in = dram.tile([N_CORES, CH, TOK // N_CORES], BF,
                               tag="a2a_in")
            a2a_out = dram.tile([N_CORES, CH, TOK // N_CORES], BF,
                                tag="a2a_out")

            ident = persist.tile([128, 128], BF, tag="ident")
            make_identity(nc, ident[:])
            ones = persist.tile([128, 128], BF, tag="ones")
            nc.vector.memset(ones[:], 1.0)
            tg_sb = persist.tile([128, H_LOC], F32, tag="tg_sb")
            nc.scalar.dma_start(tg_sb[:], tg.ap().partition_broadcast(128))
            akT_sb = persist.tile([128, H_LOC, AL], BF, tag="akT_sb")
            nc.scalar.dma_start(
                akT_sb[:], akT.ap().rearrange("p (h a) -> p h a", h=H_LOC))
            av_sb = persist.tile([AL, CH], BF, tag="av_sb")
            nc.scalar.dma_start(av_sb[:], av.ap())
            m01_sb = persist.tile([128, 4, 512], BF, tag="m01_sb")
            nc.scalar.dma_start(
                m01_sb[:], m01.ap().rearrange("p (j q) -> p j q", j=4))
            # RoPE tables in the baseline layout: [128 part, S//128, 64]
            cs_all = persist.tile([128, S // 128, HD // 2], F32, tag="cs_all")
            nc.scalar.dma_start(
                cs_all[:], fcos.ap().rearrange("(pb p) f -> p pb f", p=128))
            sn_all = persist.tile([128, S // 128, HD // 2], F32, tag="sn_all")
            nc.scalar.dma_start(
                sn_all[:], fsin.ap().rearrange("(pb p) f -> p pb f", p=128))

            # Phase B/C loads on the otherwise-idle gpsimd queue: they are
            # issued up front in its FIFO and fire as soon as the producing
            # spills' semaphores allow, prefetching across phase boundaries.
            def _bh_loads(b_i, h):
                qTb = at.tile([128, S], BF, tag="qTb", name="qTb")
                nc.gpsimd.dma_start(
                    qTb[:], qT_d[ds(h * HD, HD), ds(b_i * S, S)])
                kTb = at.tile([128, S], BF, tag="kTb", name="kTb")
                nc.gpsimd.dma_start(
                    kTb[:], kT_d[ds(h * HD, HD), ds(b_i * S, S)])
                vb2 = at.tile([128, S // 128, HD], BF, tag="vb2", name="vb2")
                nc.gpsimd.dma_start(
                    vb2[:],
                    v_d[ds(b_i * S, S), ts(h, HD)].rearrange(
                        "(kt p) d -> p kt d", p=128))
                return qTb, kTb, vb2

            # ================= phase A: QKV =================
            NPA = 16                  # 256-token panels
            with tc.tile_pool(name="wres", bufs=1) as wres, \
                 tc.tile_pool(name="xa", bufs=6) as xa, \
                 tc.tile_pool(name="ar", bufs=3) as ar, \
                 tc.tile_pool(name="aspill", bufs=2) as aspill, \
                 tc.tile_pool(name="ps_a", bufs=2, space="PSUM") as ps_a, \
                 tc.tile_pool(name="ps_t", bufs=2, space="PSUM") as ps_t:
                wTs = []
                for nm, wt in (("q", wqT), ("k", wkT), ("v", wvT)):
                    wT = wres.tile([128, NK, CH], BF, tag=f"wT{nm}",
                                   name=f"wT{nm}")
                    nc.scalar.dma_start(
                        wT[:], wt.ap().rearrange("(nk p) c -> p nk c", p=128))
                    wTs.append(wT)

                for pan in range(NPA):
                    # xT quarter-tiles for this panel: [128, 8, 256] each
                    xh = []
                    for qd in range(4):
                        xt = xa.tile([128, NK // 4, 256], BF, tag="xt")
                        nc.sync.dma_start(
                            xt[:],
                            xT.ap()[ds(qd * (D // 4), D // 4),
                                    ds(pan * 256, 256)].rearrange(
                                "(k p) t -> p k t", p=128))
                        xh.append(xt)
                    for ck in range(2):          # 128-token chunks in panel
                        tglob = pan * 2 + ck     # global token tile
                        srow = (tglob % (S // 128)) * 128  # within batch
                        csb = cs_all[:, srow // 128, :]
                        ssb = sn_all[:, srow // 128, :]
                        for p_i in range(3):     # chain-major: one PSUM bank
                            pp = ps_a.tile([128, CH], F32, tag=f"pp{p_i}",
                                           name=f"pp{p_i}")
                            for dt in range(NK):
                                lx = xh[dt // (NK // 4)][:, dt % (NK // 4),
                                                         ts(ck, 128)]
                                nc.tensor.matmul(
                                    pp[:], lhsT=lx, rhs=wTs[p_i][:, dt, :],
                                    start=(dt == 0), stop=(dt == NK - 1))
                            if p_i == 2:
                                # v: cast + spill natural
                                vb = ar.tile([128, CH], BF, tag="vb")
                                nc.scalar.activation(vb[:], pp[:], COPY)
                                nc.scalar.dma_start(
                                    v_d[ds(tglob * 128, 128), :], vb[:])
                                continue
                            # q, k: RoPE from PSUM -> natural bf16
                            dst = qT_d if p_i == 0 else kT_d
                            rp = ar.tile([128, CH], BF, tag=f"rp{p_i}",
                                         name=f"rp{p_i}")
                            for h in range(H_LOC):
                                pv2 = pp[:, ts(h, HD)].rearrange(
                                    "p (i two) -> p two i", two=2)
                                rv = rp[:, ts(h, HD)].rearrange(
                                    "p (i two) -> p two i", two=2)
                                a0, b0 = pv2[:, 0, :], pv2[:, 1, :]
                                t1 = ar.tile([128, HD // 2], F32, tag="t1")
                                t2 = ar.tile([128, HD // 2], F32, tag="t2")
                                nc.vector.tensor_mul(t1[:], a0, csb)
                                nc.vector.tensor_mul(t2[:], b0, ssb)
                                nc.vector.tensor_sub(rv[:, 0, :], t1[:], t2[:])
                                nc.vector.tensor_mul(t1[:], a0, ssb)
                                nc.vector.tensor_mul(t2[:], b0, csb)
                                nc.vector.tensor_add(rv[:, 1, :], t1[:], t2[:])
                            # transpose each 128-ch block to [ch, tok]
                            for ct in range(4):
                                tp = ps_t.tile([128, 128], BF, tag="tp")
                                nc.tensor.transpose(tp[:], rp[:, ts(ct, 128)],
                                                    ident[:])
                                st = aspill.tile([128, 128], BF, tag="st",
                                                 bufs=4)
                                nc.scalar.activation(st[:], tp[:], COPY)
                                nc.scalar.dma_start(
                                    dst[ds(ct * 128, 128),
                                        ds(tglob * 128, 128)], st[:])

            # ================= phase B: attention =================
            with tc.tile_pool(name="att", bufs=3) as att, \
                 tc.tile_pool(name="ps_st", bufs=4, space="PSUM") as ps_st, \
                 tc.tile_pool(name="ps_ac", bufs=1, space="PSUM") as ps_ac:
                cur = _bh_loads(0, 0)
                for bh in range(B * H_LOC):
                    b_i, h = divmod(bh, H_LOC)
                    nxt = (_bh_loads(*divmod(bh + 1, H_LOC))
                           if bh + 1 < B * H_LOC else None)
                    qTb, kTb, vb2 = cur
                    for qc in range(NQC):
                        nkt = (qc + 1) * 4
                        stb = att.tile([128, S // 128, 512], BF, tag="stb",
                                       bufs=2)
                        sadd = att.tile([128, 2 * NQC, 512], BF, tag="sadd",
                                        bufs=2)
                        s2 = ps_ac.tile([33, 512], F32, tag="s2")
                        o_ps = ps_ac.tile([128, 512], F32, tag="o_ps", bufs=2)

                        def _score(kt):
                            sps = ps_st.tile([128, 512], F32, tag="sps")
                            nc.tensor.matmul(sps[:],
                                             lhsT=kTb[:, ts(kt, 128)],
                                             rhs=qTb[:, ts(qc, 512)],
                                             start=True, stop=True)
                            nc.scalar.activation(stb[:, kt, :], sps[:],
                                                 EXP, scale=SCALE)
                            if kt // 4 == qc:
                                nc.vector.tensor_mul(
                                    stb[:, kt, :], stb[:, kt, :],
                                    m01_sb[:, kt % 4, :])

                        def _pv(kt):
                            nc.tensor.matmul(o_ps[:], lhsT=vb2[:, kt, :],
                                             rhs=stb[:, kt, :],
                                             start=(kt == 0),
                                             stop=(kt == nkt - 1))
                            if kt % 4 == 3:
                                g = kt // 4
                                nc.vector.tensor_add(
                                    sadd[:, 2 * g:2 * g + 2, :].rearrange(
                                        "p b a -> p (b a)"),
                                    stb[:, 4 * g:4 * g + 2, :].rearrange(
                                        "p b a -> p (b a)"),
                                    stb[:, 4 * g + 2:4 * g + 4, :].rearrange(
                                        "p b a -> p (b a)"))
                                nc.tensor.matmul(s2[0:1, :],
                                                 lhsT=ones[:, 0:1],
                                                 rhs=sadd[:, 2 * g, :],
                                                 start=(g == 0), stop=False)
                                nc.tensor.matmul(s2[0:1, :],
                                                 lhsT=ones[:, 0:1],
                                                 rhs=sadd[:, 2 * g + 1, :],
                                                 start=False,
                                                 stop=(g == nkt // 4 - 1))

                        _score(0)
                        for kt in range(1, nkt):
                            _score(kt)
                            if kt >= 2:
                                _pv(kt - 2)
                        _pv(nkt - 2)
                        _pv(nkt - 1)
                        # adapter scores [AL, 512]
                        spa = ps_st.tile([128, 512], F32, tag="sps")
                        nc.tensor.matmul(spa[:AL, :], lhsT=akT_sb[:, h, :],
                                         rhs=qTb[:, ts(qc, 512)],
                                         start=True, stop=True)
                        pab = att.tile([AL, 512], BF, tag="pab")
                        nc.scalar.activation(pab[:], spa[:AL, :], EXP,
                                             scale=SCALE)
                        nc.tensor.matmul(s2[32:33, :], lhsT=ones[:AL, 0:1],
                                         rhs=pab[:], start=True, stop=True)
                        oa_ps = ps_ac.tile([128, 512], F32, tag="oa_ps")
                        nc.tensor.matmul(oa_ps[:], lhsT=av_sb[:, ts(h, HD)],
                                         rhs=pab[:], start=True, stop=True)
                        # combine: o = o_main/s_main + tanh(g)*oa/s_adapt
                        rs2f = att.tile([33, 512], F32, tag="rs2f")
                        nc.vector.reciprocal_approx_fast(rs2f[:], s2[:])
                        rs2 = att.tile([33, 512], BF, tag="rs2")
                        nc.vector.tensor_copy(rs2[:], rs2f[:])
                        bc_ps = ps_st.tile([128, 512], F32, tag="sps")
                        bca_ps = ps_st.tile([128, 512], F32, tag="sps")
                        nc.tensor.matmul(bc_ps, lhsT=ones[0:1, :],
                                         rhs=rs2[0:1, :], start=True,
                                         stop=True)
                        nc.tensor.matmul(bca_ps, lhsT=ones[32:33, :],
                                         rhs=rs2[32:33, :], start=True,
                                         stop=True)
                        bcs = att.tile([128, 512], F32, tag="bcs")
                        nc.scalar.activation(bcs[:], bc_ps, COPY)
                        bcas = att.tile([128, 512], F32, tag="bcas")
                        nc.vector.tensor_copy(bcas[:], bca_ps)
                        t3 = att.tile([128, 512], F32, tag="t3")
                        nc.vector.tensor_mul(t3[:], o_ps[:], bcs[:])
                        t4 = att.tile([128, 512], F32, tag="t4")
                        nc.vector.scalar_tensor_tensor(
                            t4[:], bcas[:], tg_sb[:, ds(h, 1)], oa_ps[:],
                            op0=MULT, op1=MULT)
                        ob = att.tile([128, 512], BF, tag="ob")
                        nc.vector.tensor_add(ob[:], t3[:], t4[:])
                        nc.scalar.dma_start(
                            a2a_in[b_i * NQC + qc][ds(h * HD, HD), :], ob[:])
                    cur = nxt

            at_cm.__exit__(None, None, None)
            # ========== phase C: AllToAll + full-wo quarters ==========
            nc.gpsimd.collective_compute(
                "AllToAll", mybir.AluOpType.bypass,
                replica_groups=[list(range(N_CORES))],
                ins=[a2a_in.opt()], outs=[a2a_out.opt()])
            with tc.tile_pool(name="wof", bufs=2) as wof, \
                 tc.tile_pool(name="wy", bufs=4) as wy, \
                 tc.tile_pool(name="ps_y", bufs=2, space="PSUM") as ps_y:
                oTf = wof.tile([128, NK, TOK // N_CORES], BF, tag="oTf",
                               bufs=1)
                for sc in range(N_CORES):
                    nc.gpsimd.dma_start(
                        oTf[:, ds(sc * H_LOC, H_LOC), :],
                        a2a_out[sc].rearrange("(c p) t -> p c t", p=128))
                for dq in range(NDQ):
                    wqt = wof.tile([128, NK, DQW], BF, tag="wqt")
                    for cq in range(4):
                        nc.sync.dma_start(
                            wqt[:, ds(cq * (NK // 4), NK // 4), :],
                            woTF.ap()[ds(cq * (D // 4), D // 4),
                                      ds(dq * DQW, DQW)].rearrange(
                                "(ct p) d -> p ct d", p=128))
                    for tt in range(4):
                        yt = ps_y.tile([128, DQW], F32, tag="yt")
                        for ct in range(NK):
                            for dc in range(DQW // 512):
                                nc.tensor.matmul(
                                    yt[:, ts(dc, 512)],
                                    lhsT=oTf[:, ct, ts(tt, 128)],
                                    rhs=wqt[:, ct, ts(dc, 512)],
                                    start=(ct == 0), stop=(ct == NK - 1))
                        yf = wy.tile([128, DQW], F32, tag="yf")
                        nc.scalar.activation(yf[:], yt[:], COPY)
                        nc.scalar.dma_start(
                            out.ap()[ds(tt * 128, 128), ds(dq * DQW, DQW)],
                            yf[:])
    nc.compile()
    return nc


_NC_CACHE = None


def _prep(x, wq, wk, wv, wo, gate, adapter, freqs_cos, freqs_sin, mask):
    """Host-side layout prep. Returns per-core input maps."""
    xf = np.asarray(x, np.float32).reshape(TOK, D)
    xT = np.ascontiguousarray(xf.T).astype(BF_NP)
    wq = np.asarray(wq, np.float32)
    wk = np.asarray(wk, np.float32)
    wv = np.asarray(wv, np.float32)
    wo = np.asarray(wo, np.float32)
    g = np.tanh(np.asarray(gate, np.float32).reshape(H))
    ad = np.asarray(adapter, np.float32).reshape(AL, D)
    a_k = ad @ wk.T          # [AL, H*HD]
    a_v = ad @ wv.T
    fc = np.ascontiguousarray(np.asarray(freqs_cos, np.float32))
    fs = np.ascontiguousarray(np.asarray(freqs_sin, np.float32))
    woTF = np.ascontiguousarray(wo.T).astype(BF_NP)
    mk = np.asarray(mask, np.float32).reshape(S, S)
    # multiplicative 0/1 diagonal masks, S^T orientation: m01[j][k, q]
    m01 = np.empty((128, 4, 512), np.float32)
    for j in range(4):
        blk = mk[0:512, j * 128:(j + 1) * 128]    # [q, k] additive
        m01[:, j, :] = (blk == 0.0).T.astype(np.float32)
    m01 = np.ascontiguousarray(m01.reshape(128, 4 * 512)).astype(BF_NP)

    in_maps = []
    for r in range(N_CORES):
        sl = slice(r * CH, (r + 1) * CH)
        akr = a_k[:, sl]     # [AL, CH]
        akT = np.zeros((HD, H_LOC, AL), np.float32)
        for h in range(H_LOC):
            akT[:, h, :] = akr[:, h * HD:(h + 1) * HD].T
        in_maps.append({
            "xT": xT,
            "wqT": np.ascontiguousarray(wq[sl].T).astype(BF_NP),
            "wkT": np.ascontiguousarray(wk[sl].T).astype(BF_NP),
            "wvT": np.ascontiguousarray(wv[sl].T).astype(BF_NP),
            "woTF": woTF,
            "fcos": fc,
            "fsin": fs,
            "akT": np.ascontiguousarray(
                akT.reshape(HD, H_LOC * AL)).astype(BF_NP),
            "av": np.ascontiguousarray(a_v[:, sl]).astype(BF_NP),
            "tg": np.ascontiguousarray(
                g[r * H_LOC:(r + 1) * H_LOC].reshape(1, H_LOC)),
            "m01": m01,
        })
    return in_maps


def kernel(x, wq, wk, wv, wo, gate, adapter, freqs_cos, freqs_sin, mask,
           start_pos=0, **_unused):
    global _NC_CACHE
    if _NC_CACHE is None:
        _NC_CACHE = build()
    nc = _NC_CACHE
    in_maps = _prep(x, wq, wk, wv, wo, gate, adapter,
                    freqs_cos, freqs_sin, mask)
    res = bass_utils.run_bass_kernel_spmd(nc, in_maps,
                                          core_ids=list(range(N_CORES)))
    y = np.concatenate([res.results[r]["out"] for r in range(N_CORES)], axis=0)
    return y.reshape(B, S, D)


if __name__ == "__main__":
    nc = build()
    print("compiled ok, instrs:",
          sum(len(bb.instructions) for f in nc.m.functions for bb in f.blocks))
